# revision 53
# baseline (speedup 1.0000x reference)
"""Multi-Head Latent Attention (MLA) Trainium2 kernel, 8-core SPMD.

Sharding: core c -> batch b = c // 4, head-group g = c % 4 (4 heads each).

v3 structure:
 - Phase A produces TRANSPOSED latents directly on the PE (stationary = wlat
   column block, moving = xT chunk); kv blocks first so the small gkv
   AllGather (normalized kv latent + rotated shared krope) fires early.
 - The q latent never leaves the core: each core decompresses q_nope and
   rotated q_rope for ALL 16 heads on its own S chunk (the transposed latent
   tiles feed the decompress directly as moving operands), applies its local
   rmsnorm scale during PSUM eviction, and exchanges per-head payloads with
   two pipelined AllToAlls (head pair 0 first, so SDPA on heads 0/1 starts
   while pair 1 is still in flight).
 - Rope rotation runs in the transposed layout: banded cos/sin tables
   (vector muls) + a +-1 band-combine matmul pair on the PE.
 - SDPA computes scores transposed ([sk, sq]); denominators via ones-matmul.
 - Output projection is column-parallel via per-core w_proj column slices.
"""

import sys

for _p in ("/opt/trn_rl_repo", "/opt/pypackages"):
    if _p not in sys.path:
        sys.path.append(_p)

import numpy as np
import ml_dtypes

B, S, D = 2, 2048, 2048
H, HD, RD, ND = 16, 128, 64, 64
QR, KVR = 1536, 512
EPS = 1e-6
G = 4            # cores per batch group
NC = 8
HC = H // G      # heads per core
SC = S // G      # latent-path S chunk per core
NT = S // 128    # 16 s-tiles
NW = S // 512    # 4  sq windows
LATW = QR + KVR + RD   # 2112 = packed cq|ckv|krope width
SCALE = 1.0 / float(np.sqrt(HD))
NEG = -30000.0   # additive mask; * SCALE stays << exp underflow

BF = ml_dtypes.bfloat16

_cached = {}


def _build():
    import concourse.bass as bass
    import concourse.bass_isa as bass_isa
    import concourse.mybir as mybir
    import concourse.tile as tile
    from concourse import bacc
    from contextlib import ExitStack

    f32 = mybir.dt.float32
    bf16 = mybir.dt.bfloat16
    AF = mybir.ActivationFunctionType

    nc = bacc.Bacc()

    # ---- parameters (host-prepped) ----
    P_xT = nc.declare_dram_parameter("xT", [D, SC], bf16, isOutput=False)
    P_wlat = nc.declare_dram_parameter("wlat", [D, LATW], bf16, isOutput=False)
    # all-heads decompress weights, (pair, dest-core) packed columns
    P_wdqn = nc.declare_dram_parameter("wdqn", [QR, H * ND], bf16, isOutput=False)
    P_wdqr = nc.declare_dram_parameter("wdqr", [QR, H * RD], bf16, isOutput=False)
    P_wdkn = nc.declare_dram_parameter("wdkn", [KVR, HC * ND], bf16, isOutput=False)
    P_wdv = nc.declare_dram_parameter("wdv", [KVR, HC * HD], bf16, isOutput=False)
    P_wproj = nc.declare_dram_parameter("wproj", [H * HD, SC], bf16, isOutput=False)
    # banded transposed rope tables for OWN chunk [128, SC] (see _prep_inputs)
    P_tabA = nc.declare_dram_parameter("tabA", [128, SC], bf16, isOutput=False)
    P_tabB = nc.declare_dram_parameter("tabB", [128, SC], bf16, isOutput=False)
    # krope rotation tables for own chunk (transposed, banded): [64, SC]
    P_tabKA = nc.declare_dram_parameter("tabKA", [64, SC], bf16, isOutput=False)
    P_tabKB = nc.declare_dram_parameter("tabKB", [64, SC], bf16, isOutput=False)
    # rope band-combine selector matrices (+-1), see _prep_inputs
    P_rotM1 = nc.declare_dram_parameter("rotM1", [128, 128], bf16, isOutput=False)
    P_rotM2 = nc.declare_dram_parameter("rotM2", [128, 128], bf16, isOutput=False)
    # batch masks: mA = 1 if this core is in batch 0 else 0; mB = 1 - mA
    P_mA = nc.declare_dram_parameter("mA", [128, 1], f32, isOutput=False)
    P_mB = nc.declare_dram_parameter("mB", [128, 1], f32, isOutput=False)
    P_mask = nc.declare_dram_parameter("maskT", [128, 128], f32, isOutput=False)
    P_out = nc.declare_dram_parameter("out", [S, SC], f32, isOutput=True)

    groups = [[0, 1, 2, 3], [4, 5, 6, 7]]

    with ExitStack() as top:
        tc = top.enter_context(tile.TileContext(nc))

        dram = top.enter_context(tc.tile_pool(name="dram", bufs=1, space="DRAM"))
        KVW = KVR + RD   # 576
        gkv_in = dram.tile([KVW, SC], bf16, tag="gkv_in", name="gkv_in")
        gkv_out = dram.tile([G, KVW, SC], bf16, tag="gkv_out", name="gkv_out")
        # AllToAll payloads over ALL 8 cores: per head-pair, per dest core:
        # [nope(128); rope(128)].  Each sender writes its payload to BOTH
        # batch-halves' blocks, masked by mA/mB (wrong-batch copy is zeros);
        # receivers sum block g2 and block 4+g2.  Keeps addressing static.
        a2a_in = [dram.tile([NC, 256, SC], bf16, tag=f"a2a_in{p}", name=f"a2a_in{p}")
                  for p in range(2)]
        a2a_out = [dram.tile([NC, 256, SC], bf16, tag=f"a2a_out{p}", name=f"a2a_out{p}")
                   for p in range(2)]
        agh_in = [dram.tile([HD, S], bf16, tag=f"agh_in{h}", name=f"agh_in{h}")
                  for h in range(HC)]
        agh_out = [dram.tile([G, HD, S], bf16, tag=f"agh_out{h}", name=f"agh_out{h}")
                   for h in range(HC)]

        const = top.enter_context(tc.tile_pool(name="const", bufs=1))
        ones_sb = const.tile([128, 128], bf16, tag="ones", name="ones")
        nc.vector.memset(ones_sb[:], 1.0)
        onesf = const.tile([1, 128], f32, tag="onesf", name="onesf")
        nc.vector.memset(onesf[:], 1.0)
        mask_sb = const.tile([128, 128], f32, tag="mask", name="mask")
        nc.sync.dma_start(mask_sb[:], P_mask[:])
        eps_sb = const.tile([128, 1], f32, tag="eps", name="eps")
        nc.vector.memset(eps_sb[:], EPS)
        rotM1_sb = const.tile([128, 128], bf16, tag="rotM1", name="rotM1")
        nc.sync.dma_start(rotM1_sb[:], P_rotM1[:])
        rotM2_sb = const.tile([128, 128], bf16, tag="rotM2", name="rotM2")
        nc.sync.dma_start(rotM2_sb[:], P_rotM2[:])
        mA_sb = const.tile([128, 1], f32, tag="mA", name="mA")
        nc.sync.dma_start(mA_sb[:], P_mA[:])
        mB_sb = const.tile([128, 1], f32, tag="mB", name="mB")
        nc.sync.dma_start(mB_sb[:], P_mB[:])

        # tiles that must survive phase A into phase C/D
        pk = top.enter_context(tc.tile_pool(name="pk", bufs=1))
        nkvT = [pk.tile([128, S], bf16, tag=f"nkvT{rt}", name=f"nkvT{rt}")
                for rt in range(KVR // 128)]
        qT = [pk.tile([128, S], bf16, tag=f"qT{h}", name=f"qT{h}")
              for h in range(HC)]
        wdkn_sb = [pk.tile([128, HC * ND], bf16, tag=f"wdkn{rt}", name=f"wdkn{rt}")
                   for rt in range(KVR // 128)]
        wdv_sb = [pk.tile([128, HC * HD], bf16, tag=f"wdv{rt}", name=f"wdv{rt}")
                  for rt in range(KVR // 128)]

        # ===== Phase A: transposed latents + local q decompress + A2A =====
        with ExitStack() as ctxA:
            pa = ctxA.enter_context(tc.tile_pool(name="pa", bufs=1))
            pa_mv = ctxA.enter_context(tc.tile_pool(name="pa_mv", bufs=3))
            pa_ps = ctxA.enter_context(
                tc.tile_pool(name="pa_ps", bufs=4, space="PSUM"))
            pa_ss = ctxA.enter_context(
                tc.tile_pool(name="pa_ss", bufs=1, space="PSUM"))

            pa_w = ctxA.enter_context(tc.tile_pool(name="pa_w", bufs=2))

            xT_sb = []
            for dt_ in range(D // 128):
                xt = pa.tile([128, SC], bf16, tag=f"xT{dt_}", name=f"xT{dt_}")
                nc.sync.dma_start(xt[:], P_xT[dt_ * 128:(dt_ + 1) * 128, :])
                xT_sb.append(xt)
            for rt in range(KVR // 128):
                nc.sync.dma_start(wdkn_sb[rt][:],
                                  P_wdkn[rt * 128:(rt + 1) * 128, :])
                nc.sync.dma_start(wdv_sb[rt][:],
                                  P_wdv[rt * 128:(rt + 1) * 128, :])

            def wl_chunk(col0, ncols):
                # stream a [D, ncols] column chunk of wlat into a ring
                tiles = []
                for dt_ in range(D // 128):
                    t = pa_w.tile([128, 256], bf16, tag=f"wlc{dt_}",
                                  name=f"wlc{dt_}")
                    nc.sync.dma_start(
                        t[:, 0:ncols],
                        P_wlat[dt_ * 128:(dt_ + 1) * 128, col0:col0 + ncols])
                    tiles.append(t)
                return tiles

            tabKA_sb = pa.tile([64, SC], bf16, tag="tabKA", name="tabKA")
            nc.sync.dma_start(tabKA_sb[:], P_tabKA[:])
            tabKB_sb = pa.tile([64, SC], bf16, tag="tabKB", name="tabKB")
            nc.sync.dma_start(tabKB_sb[:], P_tabKB[:])
            tabA_sb = pa.tile([128, SC], bf16, tag="tabA", name="tabA")
            nc.sync.dma_start(tabA_sb[:], P_tabA[:])
            tabB_sb = pa.tile([128, SC], bf16, tag="tabB", name="tabB")
            nc.sync.dma_start(tabB_sb[:], P_tabB[:])

            def lat_pair(wl, cols, psums):
                # cols: list of (local col0, ncols) matching psums
                for dt_ in range(D // 128):
                    for (col0, ncols), ps in zip(cols, psums):
                        nc.tensor.matmul(
                            ps[0:ncols, :],
                            wl[dt_][:, col0:col0 + ncols],
                            xT_sb[dt_][:],
                            start=dt_ == 0, stop=dt_ == D // 128 - 1)

            # ---- kv blocks (2 pairs) + sumsq ----
            sskv = pa_ss.tile([1, SC], f32, tag="sskv", name="sskv")
            kvraw = []
            kvsq = []
            for pr in range(2):
                wl = wl_chunk(1536 + pr * 256, 256)
                psA = pa_ps.tile([128, SC], f32, tag="lat_ps", name="lat_ps")
                psB = pa_ps.tile([128, SC], f32, tag="lat_ps", name="lat_ps")
                lat_pair(wl, [(0, 128), (128, 128)], [psA, psB])
                for j, ps in enumerate((psA, psB)):
                    rb = pr * 2 + j
                    raw = pa.tile([128, SC], bf16, tag=f"kvraw{rb}", name=f"kvraw{rb}")
                    nc.scalar.copy(raw[:], ps[:])
                    kvraw.append(raw)
                    sq = pa.tile([128, SC], bf16, tag=f"sqkv{rb}", name=f"sqkv{rb}")
                    nc.scalar.activation(sq[:], ps[:], AF.Square)
                    kvsq.append(sq)
            # ---- krope block: rotate (no norm) ----
            wl = wl_chunk(2048, 64)
            pkr = pa_ps.tile([128, SC], f32, tag="lat_ps", name="lat_ps")
            lat_pair(wl, [(0, 64)], [pkr])
            for rb in range(4):
                nc.tensor.matmul(sskv[:], ones_sb[:, 0:1], kvsq[rb][:],
                                 start=rb == 0, stop=rb == 3)
            kr1 = pa.tile([64, SC], bf16, tag="kr1", name="kr1")
            kr2 = pa.tile([64, SC], bf16, tag="kr2", name="kr2")
            nc.vector.tensor_mul(kr1[:], pkr[0:64, :], tabKA_sb[:])
            nc.vector.tensor_mul(kr2[:], pkr[0:64, :], tabKB_sb[:])
            rotp = pa_ss.tile([128, SC], f32, tag="rot", name="rot")
            nc.tensor.matmul(rotp[0:64, :], rotM1_sb[0:64, 64:128], kr1[:],
                             start=True, stop=False)
            nc.tensor.matmul(rotp[0:64, :], rotM2_sb[0:64, 64:128], kr2[:],
                             start=False, stop=True)
            krot = pa.tile([64, SC], bf16, tag="krot", name="krot")
            nc.scalar.copy(krot[:], rotp[0:64, :])
            nc.sync.dma_start(gkv_in[KVR:KVW, :], krot[:])

            # ---- kv norm: rsqrt(mean sq + eps), broadcast, scale, ship ----
            skv = pa.tile([1, SC], f32, tag="skv", name="skv")
            nc.scalar.activation(skv[:], sskv[:], AF.Sqrt,
                                 bias=eps_sb[0:1, :], scale=1.0 / KVR)
            rkv = pa.tile([1, SC], f32, tag="rkv", name="rkv")
            nc.vector.reciprocal(rkv[:], skv[:])
            rkvb = pa.tile([128, SC], f32, tag="rkvb", name="rkvb")
            nc.gpsimd.partition_broadcast(rkvb[:], rkv[:], channels=128)
            for rb in range(4):
                kvn = pa_mv.tile([128, SC], bf16, tag="kvn", name="kvn")
                nc.vector.tensor_mul(kvn[:], kvraw[rb][:], rkvb[:])
                nc.sync.dma_start(
                    gkv_in[rb * 128:(rb + 1) * 128, :], kvn[:])

            nc.gpsimd.collective_compute(
                "AllGather", mybir.AluOpType.bypass,
                replica_groups=groups,
                ins=[gkv_in.opt()], outs=[gkv_out.opt()])

            # kv-latent + shared-krope loads: issued on the gpsimd queue right
            # behind the AG trigger so they run the moment the AG lands.
            for rt in range(KVR // 128):
                nc.gpsimd.dma_start(
                    nkvT[rt][:].rearrange("p (g c) -> p g c", g=G),
                    gkv_out[:, rt * 128:(rt + 1) * 128, :].rearrange(
                        "g p c -> p g c"))
            for h in range(HC):
                roff = 64 if h % 2 == 0 else 0   # even: [nope|rope], odd: [rope|nope]
                nc.gpsimd.dma_start(
                    qT[h][roff:roff + 64, :].rearrange(
                        "p (g c) -> p g c", g=G),
                    gkv_out[:, KVR:KVW, :].rearrange("g p c -> p g c"))

            # ---- q blocks (6 pairs), raw latents stay in SBUF ----
            # (q decompress weights stream in alongside, 2 tiles per pair)
            wdqnA = [pa.tile([128, H * ND], bf16, tag=f"wdqnA{rt}",
                             name=f"wdqnA{rt}") for rt in range(QR // 128)]
            wdqrA = [pa.tile([128, H * RD], bf16, tag=f"wdqrA{rt}",
                             name=f"wdqrA{rt}") for rt in range(QR // 128)]
            ssq = pa_ss.tile([1, SC], f32, tag="ssq", name="ssq")
            latq = []
            pend = []   # delayed sumsq ones-matmuls: (sq_tile, rb)
            for pr in range(6):
                wl = wl_chunk(pr * 256, 256)
                for rt in (2 * pr, 2 * pr + 1):
                    nc.sync.dma_start(wdqnA[rt][:],
                                      P_wdqn[rt * 128:(rt + 1) * 128, :])
                    nc.sync.dma_start(wdqrA[rt][:],
                                      P_wdqr[rt * 128:(rt + 1) * 128, :])
                psA = pa_ps.tile([128, SC], f32, tag="lat_ps", name="lat_ps")
                psB = pa_ps.tile([128, SC], f32, tag="lat_ps", name="lat_ps")
                lat_pair(wl, [(0, 128), (128, 128)], [psA, psB])
                while pend:
                    t, prb = pend.pop(0)
                    nc.tensor.matmul(ssq[:], ones_sb[:, 0:1], t[:],
                                     start=prb == 0, stop=False)
                for j, ps in enumerate((psA, psB)):
                    rb = pr * 2 + j
                    raw = pa.tile([128, SC], bf16, tag=f"latq{rb}", name=f"latq{rb}")
                    nc.scalar.copy(raw[:], ps[:])
                    latq.append(raw)
                    sq = pa_mv.tile([128, SC], bf16, tag="sq", name="sq")
                    nc.scalar.activation(sq[:], ps[:], AF.Square)
                    pend.append((sq, rb))
            for t, prb in pend:
                nc.tensor.matmul(ssq[:], ones_sb[:, 0:1], t[:],
                                 start=False, stop=prb == 11)
            pend = []

            # ---- local q rmsnorm scale ----
            sqr = pa.tile([1, SC], f32, tag="sqr", name="sqr")
            nc.scalar.activation(sqr[:], ssq[:], AF.Sqrt,
                                 bias=eps_sb[0:1, :], scale=1.0 / QR)
            rqr = pa.tile([1, SC], f32, tag="rqr", name="rqr")
            nc.vector.reciprocal(rqr[:], sqr[:])
            rqb = pa.tile([128, SC], f32, tag="rqb", name="rqb")
            nc.gpsimd.partition_broadcast(rqb[:], rqr[:], channels=128)
            tabArq = pa.tile([128, SC], bf16, tag="tabArq", name="tabArq")
            nc.vector.tensor_mul(tabArq[:], tabA_sb[:], rqb[:])
            tabBrq = pa.tile([128, SC], bf16, tag="tabBrq", name="tabBrq")
            nc.vector.tensor_mul(tabBrq[:], tabB_sb[:], rqb[:])
            rqbA = pa.tile([128, SC], bf16, tag="rqbA", name="rqbA")
            nc.vector.tensor_scalar_mul(rqbA[:], rqb[:], mA_sb[:])
            rqbB = pa.tile([128, SC], bf16, tag="rqbB", name="rqbB")
            nc.vector.tensor_scalar_mul(rqbB[:], rqb[:], mB_sb[:])

            # ---- local q decompress (all 16 heads, own chunk) + A2A ----
            for p in range(2):          # head pair within each dest
                for g2 in range(G):     # dest core, one at a time
                    psn = pa_ps.tile([128, SC], f32, tag="lat_ps", name="lat_ps")
                    psr = pa_ps.tile([128, SC], f32, tag="lat_ps", name="lat_ps")
                    col = (p * G + g2) * 128
                    for rt in range(QR // 128):
                        nc.tensor.matmul(
                            psn[:], wdqnA[rt][:, col:col + 128], latq[rt][:],
                            start=rt == 0, stop=rt == QR // 128 - 1)
                        nc.tensor.matmul(
                            psr[:], wdqrA[rt][:, col:col + 128], latq[rt][:],
                            start=rt == 0, stop=rt == QR // 128 - 1)
                    plnA = pa_mv.tile([128, SC], bf16, tag="plnA", name="plnA")
                    nc.vector.tensor_mul(plnA[0:64, :], psn[0:64, :],
                                         rqbA[0:64, :])
                    nc.vector.tensor_mul(plnA[64:128, :], psn[64:128, :],
                                         rqbA[64:128, :])
                    nc.sync.dma_start(a2a_in[p][g2, 0:128, :], plnA[:])
                    plnB = pa_mv.tile([128, SC], bf16, tag="plnB", name="plnB")
                    nc.vector.tensor_mul(plnB[0:64, :], psn[0:64, :],
                                         rqbB[0:64, :])
                    nc.vector.tensor_mul(plnB[64:128, :], psn[64:128, :],
                                         rqbB[64:128, :])
                    nc.sync.dma_start(a2a_in[p][G + g2, 0:128, :], plnB[:])
                    p1 = pa_mv.tile([128, SC], bf16, tag="p1", name="p1")
                    p2 = pa_mv.tile([128, SC], bf16, tag="p2", name="p2")
                    nc.vector.tensor_mul(p1[:], psr[:], tabArq[:])
                    nc.vector.tensor_mul(p2[:], psr[:], tabBrq[:])
                    rot = pa_ss.tile([128, SC], f32, tag="rot", name="rot")
                    nc.tensor.matmul(rot[:], rotM1_sb[:], p1[:],
                                     start=True, stop=False)
                    nc.tensor.matmul(rot[:], rotM2_sb[:], p2[:],
                                     start=False, stop=True)
                    plrA = pa_mv.tile([128, SC], bf16, tag="plrA", name="plrA")
                    nc.vector.tensor_scalar_mul(plrA[:], rot[:], mA_sb[:])
                    nc.sync.dma_start(a2a_in[p][g2, 128:256, :], plrA[:])
                    plrB = pa_mv.tile([128, SC], bf16, tag="plrB", name="plrB")
                    nc.vector.tensor_scalar_mul(plrB[:], rot[:], mB_sb[:])
                    nc.sync.dma_start(a2a_in[p][G + g2, 128:256, :], plrB[:])
                nc.gpsimd.collective_compute(
                    "AllToAll", mybir.AluOpType.bypass,
                    replica_groups=[list(range(NC))],
                    ins=[a2a_in[p].opt()], outs=[a2a_out[p].opt()])

        # ================= Phase C: kv decompress + assembly =================
        persist = top.enter_context(tc.tile_pool(name="persist", bufs=1))
        wpj = []
        for ot in range(H * HD // 128):
            t = persist.tile([128, SC], bf16, tag=f"wpj{ot}", name=f"wpj{ot}")
            nc.sync.dma_start(t[:], P_wproj[ot * 128:(ot + 1) * 128, :])
            wpj.append(t)
        kT = [persist.tile([128, S], bf16, tag=f"kT{h}", name=f"kT{h}") for h in range(HC)]
        v_sb = [persist.tile([128, HC * HD], bf16, tag=f"v{t}", name=f"v{t}") for t in range(NT)]

        with ExitStack() as ctxC:
            pc = ctxC.enter_context(tc.tile_pool(name="pc", bufs=1))
            pc_ps = ctxC.enter_context(
                tc.tile_pool(name="pc_ps", bufs=4, space="PSUM"))

            # A2A payload assembly (gpsimd queue: runs as each A2A lands):
            # batch-A halves land directly in qT/kT; batch-B halves go to
            # temps added in place later (wrong-batch copy is zeros).
            asm = {}
            for p in range(2):
                h0, h1 = 2 * p, 2 * p + 1
                nc.gpsimd.dma_start(
                    qT[h0][0:64, :].rearrange("p (g c) -> p g c", g=G),
                    a2a_out[p][0:G, 0:64, :].rearrange("g p c -> p g c"))
                nc.gpsimd.dma_start(
                    qT[h1][64:128, :].rearrange("p (g c) -> p g c", g=G),
                    a2a_out[p][0:G, 64:128, :].rearrange("g p c -> p g c"))
                tnB = pc.tile([128, S], bf16, tag=f"tnB{p}", name=f"tnB{p}")
                nc.gpsimd.dma_start(
                    tnB[:].rearrange("p (g c) -> p g c", g=G),
                    a2a_out[p][G:2 * G, 0:128, :].rearrange("g p c -> p g c"))
                nc.gpsimd.dma_start(
                    kT[h1][0:64, :].rearrange("p (g c) -> p g c", g=G),
                    a2a_out[p][0:G, 128:192, :].rearrange("g p c -> p g c"))
                nc.gpsimd.dma_start(
                    kT[h0][64:128, :].rearrange("p (g c) -> p g c", g=G),
                    a2a_out[p][0:G, 192:256, :].rearrange("g p c -> p g c"))
                trB = pc.tile([128, S], bf16, tag=f"trB{p}", name=f"trB{p}")
                nc.gpsimd.dma_start(
                    trB[:].rearrange("p (g c) -> p g c", g=G),
                    a2a_out[p][G:2 * G, 128:256, :].rearrange("g p c -> p g c"))
                asm[p] = (tnB, trB)

            def asm_add(p):
                h0, h1 = 2 * p, 2 * p + 1
                tnB, trB = asm[p]
                nc.vector.tensor_add(qT[h0][0:64, :], qT[h0][0:64, :],
                                     tnB[0:64, :])
                nc.vector.tensor_add(qT[h1][64:128, :], qT[h1][64:128, :],
                                     tnB[64:128, :])
                nc.vector.tensor_add(kT[h1][0:64, :], kT[h1][0:64, :],
                                     trB[0:64, :])
                nc.vector.tensor_add(kT[h0][64:128, :], kT[h0][64:128, :],
                                     trB[64:128, :])

            asm_add(0)
            # ---- v (natural layout), two interleaved chains ----
            for st2 in range(NT // 2):
                pv = [pc_ps.tile([128, HC * HD], f32, tag="dec_ps", name="dec_ps")
                      for _ in range(2)]
                for rt in range(KVR // 128):
                    for j in range(2):
                        st = st2 * 2 + j
                        nc.tensor.matmul(
                            pv[j][:], nkvT[rt][:, st * 128:(st + 1) * 128],
                            wdv_sb[rt][:],
                            start=rt == 0, stop=rt == KVR // 128 - 1)
                for j in range(2):
                    nc.scalar.copy(v_sb[st2 * 2 + j][:], pv[j][:])

            # ---- k_nope: head-pair packed, transposed layout ----
            def knope_pair(p):
                psl = [pc_ps.tile([128, 512], f32, tag="dec_ps", name="dec_ps")
                       for _ in range(S // 512)]
                for rt in range(KVR // 128):
                    stat = wdkn_sb[rt][:, p * 128:(p + 1) * 128]
                    for sc4 in range(S // 512):
                        nc.tensor.matmul(
                            psl[sc4][:], stat,
                            nkvT[rt][:, sc4 * 512:(sc4 + 1) * 512],
                            start=rt == 0, stop=rt == KVR // 128 - 1)
                h0, h1 = 2 * p, 2 * p + 1
                for sc4 in range(S // 512):
                    sl = slice(sc4 * 512, (sc4 + 1) * 512)
                    nc.vector.tensor_copy(kT[h0][0:64, sl], psl[sc4][0:64, :])
                    nc.vector.tensor_copy(kT[h1][64:128, sl], psl[sc4][64:128, :])

            knope_pair(0)
            asm_add(1)
            knope_pair(1)

        # ================= Phase D: causal SDPA (4 heads) =================
        with ExitStack() as ctxD:
            pd_mv = ctxD.enter_context(tc.tile_pool(name="pd_mv", bufs=4))
            pd_probs = ctxD.enter_context(tc.tile_pool(name="pd_probs", bufs=6))
            pd_sc = ctxD.enter_context(
                tc.tile_pool(name="pd_sc", bufs=4, space="PSUM"))
            pd_acc = ctxD.enter_context(
                tc.tile_pool(name="pd_acc", bufs=2, space="PSUM"))

            for h in range(HC):
                vcol = slice(h * HD, (h + 1) * HD)
                for w in range(NW):
                    nk = 4 * (w + 1)
                    # probs accumulate on gpsimd; denominator = partition sum
                    dacc = pd_mv.tile([128, 512], f32, tag="dacc", name="dacc")
                    att = pd_acc.tile([128, 512], f32, tag="att", name="att")
                    for kt in range(nk):
                        off = max(0, 128 * kt - 512 * w)
                        sq0 = 512 * w + off
                        ssc = pd_sc.tile([128, 512], f32, tag="ssc", name="ssc")
                        nc.tensor.matmul(
                            ssc[:, off:512],
                            kT[h][:, kt * 128:(kt + 1) * 128],
                            qT[h][:, sq0:512 * (w + 1)],
                            start=True, stop=True)
                        if kt >= 4 * w:   # block containing the diagonal
                            nc.vector.tensor_add(
                                ssc[:, off:off + 128],
                                ssc[:, off:off + 128], mask_sb[:])
                        probs = pd_probs.tile([128, 512], bf16, tag="probs", name="probs")
                        if kt == 0:
                            nc.scalar.activation(
                                probs[:], ssc[:], AF.Exp, scale=SCALE)
                            nc.gpsimd.tensor_copy(dacc[:], probs[:])
                        else:
                            nc.scalar.activation(
                                probs[:, off:512], ssc[:, off:512],
                                AF.Exp, scale=SCALE)
                            nc.gpsimd.tensor_add(dacc[:, off:512],
                                                 dacc[:, off:512],
                                                 probs[:, off:512])
                        nc.tensor.matmul(
                            att[:, off:512], v_sb[kt][:, vcol],
                            probs[:, off:512],
                            start=kt == 0, stop=kt == nk - 1)
                    den = pd_mv.tile([128, 512], f32, tag="dpar", name="dpar")
                    nc.gpsimd.partition_all_reduce(
                        den[:], dacc[:], channels=128,
                        reduce_op=bass_isa.ReduceOp.add)
                    rec = pd_mv.tile([128, 512], f32, tag="rec", name="rec")
                    nc.vector.reciprocal(rec[:], den[:])
                    outT = pd_mv.tile([128, 512], bf16, tag="outT", name="outT")
                    nc.vector.tensor_mul(outT[:], att[:], rec[:])
                    nc.sync.dma_start(
                        agh_in[h][:, w * 512:(w + 1) * 512], outT[:])
                nc.gpsimd.collective_compute(
                    "AllGather", mybir.AluOpType.bypass,
                    replica_groups=groups,
                    ins=[agh_in[h].opt()], outs=[agh_out[h].opt()])

        # ===== column-parallel projection (attn-out AGs issued per head) ====
        with ExitStack() as ctxE:
            pe = ctxE.enter_context(tc.tile_pool(name="pe", bufs=1))
            pe_mv = ctxE.enter_context(tc.tile_pool(name="pe_mv", bufs=4))
            pe_ps = ctxE.enter_context(
                tc.tile_pool(name="pe_ps", bufs=5, space="PSUM"))

            aT = [None] * (H * HD // 128)
            for hc in range(HC):          # arrival order: head-chunk 0..3
                for g2 in range(G):
                    ot = 4 * g2 + hc      # global o-tile (= global head)
                    t = pe.tile([128, S], bf16, tag=f"aT{ot}", name=f"aT{ot}")
                    nc.sync.dma_start(t[:], agh_out[hc][g2, :, :])
                    aT[ot] = t
            order = [(hc, g2) for hc in range(HC) for g2 in range(G)]
            for st4 in range(NT // 2):
                pp = [pe_ps.tile([128, SC], f32, tag="proj_ps", name="proj_ps")
                      for _ in range(2)]
                for i, (hc, g2) in enumerate(order):
                    ot = 4 * g2 + hc
                    for j in range(2):
                        st2 = st4 * 2 + j
                        nc.tensor.matmul(
                            pp[j][:], aT[ot][:, st2 * 128:(st2 + 1) * 128],
                            wpj[ot][:],
                            start=i == 0, stop=i == H * HD // 128 - 1)
                for j in range(2):
                    st2 = st4 * 2 + j
                    o_sb = pe_mv.tile([128, SC], f32, tag="o_sb", name="o_sb")
                    nc.scalar.copy(o_sb[:], pp[j][:])
                    nc.sync.dma_start(
                        P_out[st2 * 128:(st2 + 1) * 128, :], o_sb[:])

    nc.compile()
    return nc


def _get_nc():
    if "nc" not in _cached:
        _cached["nc"] = _build()
    return _cached["nc"]


def _prep_inputs(inputs):
    x = np.asarray(inputs["x"], np.float32)
    fc = np.asarray(inputs["freqs_cos"], np.float32)   # [S, 32]
    fs = np.asarray(inputs["freqs_sin"], np.float32)
    w_cq = np.asarray(inputs["w_cq"], np.float32)
    w_dq_nope = np.asarray(inputs["w_dq_nope"], np.float32)
    w_dq_rope = np.asarray(inputs["w_dq_rope"], np.float32)
    w_ckv = np.asarray(inputs["w_ckv"], np.float32)
    w_dk_nope = np.asarray(inputs["w_dk_nope"], np.float32)
    w_dv = np.asarray(inputs["w_dv"], np.float32)
    w_krope = np.asarray(inputs["w_krope"], np.float32)
    w_proj = np.asarray(inputs["w_proj"], np.float32)
    qw = np.asarray(inputs["q_norm_w"], np.float32)
    kvw = np.asarray(inputs["kv_norm_w"], np.float32)

    perm = np.concatenate([np.arange(0, RD, 2), np.arange(1, RD, 2)])

    wlat = np.concatenate(
        [w_cq.T, w_ckv.T, w_krope[perm, :].T], axis=1).astype(BF)  # [D, LATW]
    wdqn = (w_dq_nope * qw[None, :]).reshape(H, ND, QR)
    wdqr = (w_dq_rope * qw[None, :]).reshape(H, RD, QR)[:, perm, :]
    wdkn = (w_dk_nope * kvw[None, :])
    wdv = (w_dv * kvw[None, :])
    wprojT = np.ascontiguousarray(w_proj.T).astype(BF)

    # all-heads decompress weights, column blocks ordered (pair, dest):
    # block (p*G+g2) covers heads (4*g2+2p, 4*g2+2p+1)
    head_order = []
    for p in range(2):
        for g2 in range(G):
            head_order += [4 * g2 + 2 * p, 4 * g2 + 2 * p + 1]
    wdqnA = np.ascontiguousarray(
        wdqn[head_order].reshape(H * ND, QR).T).astype(BF)
    wdqrA = np.ascontiguousarray(
        wdqr[head_order].reshape(H * RD, QR).T).astype(BF)

    # banded transposed rope tables [128, S]
    tabA = np.empty((128, S), np.float32)
    tabB = np.empty((128, S), np.float32)
    for band in range(4):
        rows = slice(band * 32, (band + 1) * 32)
        if band % 2 == 0:
            tabA[rows] = fc.T
            tabB[rows] = fs.T
        else:
            tabA[rows] = fs.T
            tabB[rows] = fc.T
    tabA = tabA.astype(BF)
    tabB = tabB.astype(BF)

    # band-combine selectors: rot = rotM1.T @ (x*tabA) + rotM2.T @ (x*tabB)
    # output 64-blocks are SWAPPED vs input blocks so that PSUM rows land on
    # the same base partitions as their kT destinations (even-head rope at
    # partitions 64:128, odd-head rope at 0:64).
    rotM1 = np.zeros((128, 128), np.float32)
    rotM2 = np.zeros((128, 128), np.float32)
    for b in range(2):
        for i in range(32):
            ei, oi = b * 64 + i, b * 64 + 32 + i            # input rows
            eo, oo = (1 - b) * 64 + i, (1 - b) * 64 + 32 + i  # output rows
            rotM1[ei, eo] = 1.0
            rotM1[oi, eo] = -1.0
            rotM2[ei, oo] = 1.0
            rotM2[oi, oo] = 1.0
    rotM1 = rotM1.astype(BF)
    rotM2 = rotM2.astype(BF)

    maskT = np.zeros((128, 128), np.float32)
    il, jl = np.tril_indices(128, -1)   # sq < sk  -> masked
    maskT[il, jl] = NEG

    in_maps = []
    for c in range(NC):
        b, g = divmod(c, G)
        hsl = slice(g * HC, (g + 1) * HC)
        ssl = slice(g * SC, (g + 1) * SC)
        xT_c = np.ascontiguousarray(x[b].T[:, ssl]).astype(BF)
        wdkn_c = np.ascontiguousarray(
            wdkn.reshape(H, ND, KVR)[hsl].reshape(HC * ND, KVR).T).astype(BF)
        wdv_c = np.ascontiguousarray(
            wdv.reshape(H, HD, KVR)[hsl].reshape(HC * HD, KVR).T).astype(BF)
        wproj_c = np.ascontiguousarray(wprojT[:, ssl])
        in_maps.append({
            "xT": xT_c,
            "wlat": wlat,
            "wdqn": wdqnA,
            "wdqr": wdqrA,
            "wdkn": wdkn_c,
            "wdv": wdv_c,
            "wproj": wproj_c,
            "tabA": np.ascontiguousarray(tabA[:, ssl]),
            "tabB": np.ascontiguousarray(tabB[:, ssl]),
            "tabKA": np.ascontiguousarray(
                np.concatenate([fc[ssl].T, fs[ssl].T], axis=0)).astype(BF),
            "tabKB": np.ascontiguousarray(
                np.concatenate([fs[ssl].T, fc[ssl].T], axis=0)).astype(BF),
            "rotM1": rotM1,
            "rotM2": rotM2,
            "mA": np.full((128, 1), 1.0 - b, np.float32),
            "mB": np.full((128, 1), float(b), np.float32),
            "maskT": maskT,
        })
    return in_maps


def _assemble(results):
    out = np.zeros((B, S, H * HD), np.float32)
    for c in range(NC):
        b, g = divmod(c, G)
        out[b, :, g * SC:(g + 1) * SC] = results[c]["out"]
    return out


def kernel(**inputs) -> np.ndarray:
    from concourse.bass_utils import run_bass_kernel_spmd
    nc = _get_nc()
    in_maps = _prep_inputs(inputs)
    res = run_bass_kernel_spmd(nc, in_maps, core_ids=list(range(NC)))
    return _assemble(res.results)


# revision 54
# speedup vs baseline: 1.5979x; 1.5979x over previous
"""Multi-Head Latent Attention (MLA) Trainium2 kernel, 8-core SPMD.

Sharding: core c -> batch b = c // 4, head-group g = c % 4 (4 heads each).

v3 structure:
 - Phase A produces TRANSPOSED latents directly on the PE (stationary = wlat
   column block, moving = xT chunk); kv blocks first so the small gkv
   AllGather (normalized kv latent + rotated shared krope) fires early.
 - The q latent never leaves the core: each core decompresses q_nope and
   rotated q_rope for ALL 16 heads on its own S chunk (the transposed latent
   tiles feed the decompress directly as moving operands), applies its local
   rmsnorm scale during PSUM eviction, and exchanges per-head payloads with
   two pipelined AllToAlls (head pair 0 first, so SDPA on heads 0/1 starts
   while pair 1 is still in flight).
 - Rope rotation runs in the transposed layout: banded cos/sin tables
   (vector muls) + a +-1 band-combine matmul pair on the PE.
 - SDPA computes scores transposed ([sk, sq]); denominators via ones-matmul.
 - Output projection is column-parallel via per-core w_proj column slices.
"""

import sys

for _p in ("/opt/trn_rl_repo", "/opt/pypackages"):
    if _p not in sys.path:
        sys.path.append(_p)

import numpy as np
import ml_dtypes

B, S, D = 2, 2048, 2048
H, HD, RD, ND = 16, 128, 64, 64
QR, KVR = 1536, 512
EPS = 1e-6
G = 4            # cores per batch group
NC = 8
HC = H // G      # heads per core
SC = S // G      # latent-path S chunk per core
NT = S // 128    # 16 s-tiles
NW = S // 512    # 4  sq windows
LATW = QR + KVR + RD   # 2112 = packed cq|ckv|krope width
SCALE = 1.0 / float(np.sqrt(HD))
NEG = -30000.0   # additive mask; * SCALE stays << exp underflow

BF = ml_dtypes.bfloat16

_cached = {}


def _build():
    import concourse.bass as bass
    import concourse.bass_isa as bass_isa
    import concourse.mybir as mybir
    import concourse.tile as tile
    from concourse import bacc
    from contextlib import ExitStack

    f32 = mybir.dt.float32
    bf16 = mybir.dt.bfloat16
    AF = mybir.ActivationFunctionType

    nc = bacc.Bacc()

    # ---- parameters (host-prepped) ----
    P_xT = nc.declare_dram_parameter("xT", [D, SC], bf16, isOutput=False)
    P_wlat = nc.declare_dram_parameter("wlat", [D, LATW], bf16, isOutput=False)
    # all-heads decompress weights, (pair, dest-core) packed columns
    P_wdqn = nc.declare_dram_parameter("wdqn", [QR, H * ND], bf16, isOutput=False)
    P_wdqr = nc.declare_dram_parameter("wdqr", [QR, H * RD], bf16, isOutput=False)
    P_wdkn = nc.declare_dram_parameter("wdkn", [KVR, HC * ND], bf16, isOutput=False)
    P_wdv = nc.declare_dram_parameter("wdv", [KVR, HC * HD], bf16, isOutput=False)
    P_wproj = nc.declare_dram_parameter("wproj", [H * HD, SC], bf16, isOutput=False)
    # banded transposed rope tables for OWN chunk [128, SC] (see _prep_inputs)
    P_tabA = nc.declare_dram_parameter("tabA", [128, SC], bf16, isOutput=False)
    P_tabB = nc.declare_dram_parameter("tabB", [128, SC], bf16, isOutput=False)
    # krope rotation tables for own chunk (transposed, banded): [64, SC]
    P_tabKA = nc.declare_dram_parameter("tabKA", [64, SC], bf16, isOutput=False)
    P_tabKB = nc.declare_dram_parameter("tabKB", [64, SC], bf16, isOutput=False)
    # rope band-combine selector matrices (+-1), see _prep_inputs
    P_rotM1 = nc.declare_dram_parameter("rotM1", [128, 128], bf16, isOutput=False)
    P_rotM2 = nc.declare_dram_parameter("rotM2", [128, 128], bf16, isOutput=False)
    # batch masks: mA = 1 if this core is in batch 0 else 0; mB = 1 - mA
    P_mA = nc.declare_dram_parameter("mA", [128, 1], f32, isOutput=False)
    P_mB = nc.declare_dram_parameter("mB", [128, 1], f32, isOutput=False)
    P_mask = nc.declare_dram_parameter("maskT", [128, 128], f32, isOutput=False)
    P_out = nc.declare_dram_parameter("out", [S, SC], f32, isOutput=True)

    groups = [[0, 1, 2, 3], [4, 5, 6, 7]]

    with ExitStack() as top:
        tc = top.enter_context(tile.TileContext(nc))

        dram = top.enter_context(tc.tile_pool(name="dram", bufs=1, space="DRAM"))
        KVW = KVR + RD   # 576
        gkv_in = dram.tile([KVW, SC], bf16, tag="gkv_in", name="gkv_in")
        gkv_out = dram.tile([G, KVW, SC], bf16, tag="gkv_out", name="gkv_out")
        # AllToAll payloads over ALL 8 cores: per head-pair, per dest core:
        # [nope(128); rope(128)].  Each sender writes its payload to BOTH
        # batch-halves' blocks, masked by mA/mB (wrong-batch copy is zeros);
        # receivers sum block g2 and block 4+g2.  Keeps addressing static.
        a2a_in = [dram.tile([NC, 256, SC], bf16, tag=f"a2a_in{p}", name=f"a2a_in{p}")
                  for p in range(2)]
        a2a_out = [dram.tile([NC, 256, SC], bf16, tag=f"a2a_out{p}", name=f"a2a_out{p}")
                   for p in range(2)]
        agh_in = [dram.tile([HD, S], bf16, tag=f"agh_in{h}", name=f"agh_in{h}")
                  for h in range(HC)]
        agh_out = [dram.tile([G, HD, S], bf16, tag=f"agh_out{h}", name=f"agh_out{h}")
                   for h in range(HC)]

        const = top.enter_context(tc.tile_pool(name="const", bufs=1))
        ones_sb = const.tile([128, 128], bf16, tag="ones", name="ones")
        nc.vector.memset(ones_sb[:], 1.0)
        onesf = const.tile([1, 128], f32, tag="onesf", name="onesf")
        nc.vector.memset(onesf[:], 1.0)
        mask_sb = const.tile([128, 128], f32, tag="mask", name="mask")
        nc.sync.dma_start(mask_sb[:], P_mask[:])
        eps_sb = const.tile([128, 1], f32, tag="eps", name="eps")
        nc.vector.memset(eps_sb[:], EPS)
        rotM1_sb = const.tile([128, 128], bf16, tag="rotM1", name="rotM1")
        nc.sync.dma_start(rotM1_sb[:], P_rotM1[:])
        rotM2_sb = const.tile([128, 128], bf16, tag="rotM2", name="rotM2")
        nc.sync.dma_start(rotM2_sb[:], P_rotM2[:])
        mA_sb = const.tile([128, 1], f32, tag="mA", name="mA")
        nc.sync.dma_start(mA_sb[:], P_mA[:])
        mB_sb = const.tile([128, 1], f32, tag="mB", name="mB")
        nc.sync.dma_start(mB_sb[:], P_mB[:])

        # tiles that must survive phase A into phase C/D
        pk = top.enter_context(tc.tile_pool(name="pk", bufs=1))
        nkvT = [pk.tile([128, S], bf16, tag=f"nkvT{rt}", name=f"nkvT{rt}")
                for rt in range(KVR // 128)]
        qT = [pk.tile([128, S], bf16, tag=f"qT{h}", name=f"qT{h}")
              for h in range(HC)]
        wdkn_sb = [pk.tile([128, HC * ND], bf16, tag=f"wdkn{rt}", name=f"wdkn{rt}")
                   for rt in range(KVR // 128)]
        wdv_sb = [pk.tile([128, HC * HD], bf16, tag=f"wdv{rt}", name=f"wdv{rt}")
                  for rt in range(KVR // 128)]

        # ===== Phase A: transposed latents + local q decompress + A2A =====
        with ExitStack() as ctxA:
            pa = ctxA.enter_context(tc.tile_pool(name="pa", bufs=1))
            pa_mv = ctxA.enter_context(tc.tile_pool(name="pa_mv", bufs=3))
            pa_ps = ctxA.enter_context(
                tc.tile_pool(name="pa_ps", bufs=4, space="PSUM"))
            pa_ss = ctxA.enter_context(
                tc.tile_pool(name="pa_ss", bufs=1, space="PSUM"))

            pa_w = ctxA.enter_context(tc.tile_pool(name="pa_w", bufs=2))

            xT_sb = []
            for dt_ in range(D // 128):
                xt = pa.tile([128, SC], bf16, tag=f"xT{dt_}", name=f"xT{dt_}")
                nc.sync.dma_start(xt[:], P_xT[dt_ * 128:(dt_ + 1) * 128, :])
                xT_sb.append(xt)
            for rt in range(KVR // 128):
                nc.sync.dma_start(wdkn_sb[rt][:],
                                  P_wdkn[rt * 128:(rt + 1) * 128, :])
                nc.sync.dma_start(wdv_sb[rt][:],
                                  P_wdv[rt * 128:(rt + 1) * 128, :])

            def wl_chunk(col0, ncols):
                # stream a [D, ncols] column chunk of wlat into a ring
                tiles = []
                for dt_ in range(D // 128):
                    t = pa_w.tile([128, 256], bf16, tag=f"wlc{dt_}",
                                  name=f"wlc{dt_}")
                    nc.sync.dma_start(
                        t[:, 0:ncols],
                        P_wlat[dt_ * 128:(dt_ + 1) * 128, col0:col0 + ncols])
                    tiles.append(t)
                return tiles

            tabKA_sb = pa.tile([64, SC], bf16, tag="tabKA", name="tabKA")
            nc.sync.dma_start(tabKA_sb[:], P_tabKA[:])
            tabKB_sb = pa.tile([64, SC], bf16, tag="tabKB", name="tabKB")
            nc.sync.dma_start(tabKB_sb[:], P_tabKB[:])
            tabA_sb = pa.tile([128, SC], bf16, tag="tabA", name="tabA")
            nc.sync.dma_start(tabA_sb[:], P_tabA[:])
            tabB_sb = pa.tile([128, SC], bf16, tag="tabB", name="tabB")
            nc.sync.dma_start(tabB_sb[:], P_tabB[:])

            def lat_pair(wl, cols, psums):
                # cols: list of (local col0, ncols) matching psums
                for dt_ in range(D // 128):
                    for (col0, ncols), ps in zip(cols, psums):
                        nc.tensor.matmul(
                            ps[0:ncols, :],
                            wl[dt_][:, col0:col0 + ncols],
                            xT_sb[dt_][:],
                            start=dt_ == 0, stop=dt_ == D // 128 - 1)

            # ---- kv blocks (2 pairs) + sumsq ----
            sskv = pa_ss.tile([1, SC], f32, tag="sskv", name="sskv")
            kvraw = []
            kvsq = []
            for pr in range(2):
                wl = wl_chunk(1536 + pr * 256, 256)
                psA = pa_ps.tile([128, SC], f32, tag="lat_ps", name="lat_ps")
                psB = pa_ps.tile([128, SC], f32, tag="lat_ps", name="lat_ps")
                lat_pair(wl, [(0, 128), (128, 128)], [psA, psB])
                for j, ps in enumerate((psA, psB)):
                    rb = pr * 2 + j
                    raw = pa.tile([128, SC], bf16, tag=f"kvraw{rb}", name=f"kvraw{rb}")
                    nc.scalar.copy(raw[:], ps[:])
                    kvraw.append(raw)
                    sq = pa.tile([128, SC], bf16, tag=f"sqkv{rb}", name=f"sqkv{rb}")
                    nc.scalar.activation(sq[:], ps[:], AF.Square)
                    kvsq.append(sq)
            # ---- krope block: rotate (no norm) ----
            wl = wl_chunk(2048, 64)
            pkr = pa_ps.tile([128, SC], f32, tag="lat_ps", name="lat_ps")
            lat_pair(wl, [(0, 64)], [pkr])
            for rb in range(4):
                nc.tensor.matmul(sskv[:], ones_sb[:, 0:1], kvsq[rb][:],
                                 start=rb == 0, stop=rb == 3)
            kr1 = pa.tile([64, SC], bf16, tag="kr1", name="kr1")
            kr2 = pa.tile([64, SC], bf16, tag="kr2", name="kr2")
            nc.vector.tensor_mul(kr1[:], pkr[0:64, :], tabKA_sb[:])
            nc.vector.tensor_mul(kr2[:], pkr[0:64, :], tabKB_sb[:])
            rotp = pa_ss.tile([128, SC], f32, tag="rot", name="rot")
            nc.tensor.matmul(rotp[0:64, :], rotM1_sb[0:64, 64:128], kr1[:],
                             start=True, stop=False)
            nc.tensor.matmul(rotp[0:64, :], rotM2_sb[0:64, 64:128], kr2[:],
                             start=False, stop=True)
            krot = pa.tile([64, SC], bf16, tag="krot", name="krot")
            nc.scalar.copy(krot[:], rotp[0:64, :])
            nc.sync.dma_start(gkv_in[KVR:KVW, :], krot[:])

            # ---- kv norm: rsqrt(mean sq + eps), broadcast, scale, ship ----
            skv = pa.tile([1, SC], f32, tag="skv", name="skv")
            nc.scalar.activation(skv[:], sskv[:], AF.Sqrt,
                                 bias=eps_sb[0:1, :], scale=1.0 / KVR)
            rkv = pa.tile([1, SC], f32, tag="rkv", name="rkv")
            nc.vector.reciprocal(rkv[:], skv[:])
            rkvb = pa.tile([128, SC], f32, tag="rkvb", name="rkvb")
            nc.gpsimd.partition_broadcast(rkvb[:], rkv[:], channels=128)
            for rb in range(4):
                kvn = pa_mv.tile([128, SC], bf16, tag="kvn", name="kvn")
                nc.vector.tensor_mul(kvn[:], kvraw[rb][:], rkvb[:])
                nc.sync.dma_start(
                    gkv_in[rb * 128:(rb + 1) * 128, :], kvn[:])

            nc.gpsimd.collective_compute(
                "AllGather", mybir.AluOpType.bypass,
                replica_groups=groups,
                ins=[gkv_in.opt()], outs=[gkv_out.opt()])

            # kv-latent + shared-krope loads: issued on the gpsimd queue right
            # behind the AG trigger so they run the moment the AG lands.
            for rt in range(KVR // 128):
                nc.gpsimd.dma_start(
                    nkvT[rt][:].rearrange("p (g c) -> p g c", g=G),
                    gkv_out[:, rt * 128:(rt + 1) * 128, :].rearrange(
                        "g p c -> p g c"))
            for h in range(HC):
                roff = 64 if h % 2 == 0 else 0   # even: [nope|rope], odd: [rope|nope]
                nc.gpsimd.dma_start(
                    qT[h][roff:roff + 64, :].rearrange(
                        "p (g c) -> p g c", g=G),
                    gkv_out[:, KVR:KVW, :].rearrange("g p c -> p g c"))

            # ---- q blocks (6 pairs), raw latents stay in SBUF ----
            # (q decompress weights stream in alongside, 2 tiles per pair)
            wdqnA = [pa.tile([128, H * ND], bf16, tag=f"wdqnA{rt}",
                             name=f"wdqnA{rt}") for rt in range(QR // 128)]
            wdqrA = [pa.tile([128, H * RD], bf16, tag=f"wdqrA{rt}",
                             name=f"wdqrA{rt}") for rt in range(QR // 128)]
            ssq = pa_ss.tile([1, SC], f32, tag="ssq", name="ssq")
            latq = []
            pend = []   # delayed sumsq ones-matmuls: (sq_tile, rb)
            for pr in range(6):
                wl = wl_chunk(pr * 256, 256)
                for rt in (2 * pr, 2 * pr + 1):
                    nc.sync.dma_start(wdqnA[rt][:],
                                      P_wdqn[rt * 128:(rt + 1) * 128, :])
                    nc.sync.dma_start(wdqrA[rt][:],
                                      P_wdqr[rt * 128:(rt + 1) * 128, :])
                psA = pa_ps.tile([128, SC], f32, tag="lat_ps", name="lat_ps")
                psB = pa_ps.tile([128, SC], f32, tag="lat_ps", name="lat_ps")
                lat_pair(wl, [(0, 128), (128, 128)], [psA, psB])
                while pend:
                    t, prb = pend.pop(0)
                    nc.tensor.matmul(ssq[:], ones_sb[:, 0:1], t[:],
                                     start=prb == 0, stop=False)
                for j, ps in enumerate((psA, psB)):
                    rb = pr * 2 + j
                    raw = pa.tile([128, SC], bf16, tag=f"latq{rb}", name=f"latq{rb}")
                    nc.scalar.copy(raw[:], ps[:])
                    latq.append(raw)
                    sq = pa_mv.tile([128, SC], bf16, tag="sq", name="sq")
                    nc.scalar.activation(sq[:], ps[:], AF.Square)
                    pend.append((sq, rb))
            for t, prb in pend:
                nc.tensor.matmul(ssq[:], ones_sb[:, 0:1], t[:],
                                 start=False, stop=prb == 11)
            pend = []

            # ---- local q rmsnorm scale ----
            sqr = pa.tile([1, SC], f32, tag="sqr", name="sqr")
            nc.scalar.activation(sqr[:], ssq[:], AF.Sqrt,
                                 bias=eps_sb[0:1, :], scale=1.0 / QR)
            rqr = pa.tile([1, SC], f32, tag="rqr", name="rqr")
            nc.vector.reciprocal(rqr[:], sqr[:])
            rqb = pa.tile([128, SC], f32, tag="rqb", name="rqb")
            nc.gpsimd.partition_broadcast(rqb[:], rqr[:], channels=128)
            tabArq = pa.tile([128, SC], bf16, tag="tabArq", name="tabArq")
            nc.vector.tensor_mul(tabArq[:], tabA_sb[:], rqb[:])
            tabBrq = pa.tile([128, SC], bf16, tag="tabBrq", name="tabBrq")
            nc.vector.tensor_mul(tabBrq[:], tabB_sb[:], rqb[:])
            rqbA = pa.tile([128, SC], bf16, tag="rqbA", name="rqbA")
            nc.vector.tensor_scalar_mul(rqbA[:], rqb[:], mA_sb[:])
            rqbB = pa.tile([128, SC], bf16, tag="rqbB", name="rqbB")
            nc.vector.tensor_scalar_mul(rqbB[:], rqb[:], mB_sb[:])

            # ---- local q decompress (all 16 heads, own chunk) + A2A ----
            for p in range(2):          # head pair within each dest
                for g2 in range(G):     # dest core, one at a time
                    psn = pa_ps.tile([128, SC], f32, tag="lat_ps", name="lat_ps")
                    psr = pa_ps.tile([128, SC], f32, tag="lat_ps", name="lat_ps")
                    col = (p * G + g2) * 128
                    for rt in range(QR // 128):
                        nc.tensor.matmul(
                            psn[:], wdqnA[rt][:, col:col + 128], latq[rt][:],
                            start=rt == 0, stop=rt == QR // 128 - 1)
                        nc.tensor.matmul(
                            psr[:], wdqrA[rt][:, col:col + 128], latq[rt][:],
                            start=rt == 0, stop=rt == QR // 128 - 1)
                    plnA = pa_mv.tile([128, SC], bf16, tag="plnA", name="plnA")
                    nc.vector.tensor_mul(plnA[0:64, :], psn[0:64, :],
                                         rqbA[0:64, :])
                    nc.vector.tensor_mul(plnA[64:128, :], psn[64:128, :],
                                         rqbA[64:128, :])
                    nc.sync.dma_start(a2a_in[p][g2, 0:128, :], plnA[:])
                    plnB = pa_mv.tile([128, SC], bf16, tag="plnB", name="plnB")
                    nc.vector.tensor_mul(plnB[0:64, :], psn[0:64, :],
                                         rqbB[0:64, :])
                    nc.vector.tensor_mul(plnB[64:128, :], psn[64:128, :],
                                         rqbB[64:128, :])
                    nc.sync.dma_start(a2a_in[p][G + g2, 0:128, :], plnB[:])
                    p1 = pa_mv.tile([128, SC], bf16, tag="p1", name="p1")
                    p2 = pa_mv.tile([128, SC], bf16, tag="p2", name="p2")
                    nc.vector.tensor_mul(p1[:], psr[:], tabArq[:])
                    nc.vector.tensor_mul(p2[:], psr[:], tabBrq[:])
                    rot = pa_ss.tile([128, SC], f32, tag="rot", name="rot")
                    nc.tensor.matmul(rot[:], rotM1_sb[:], p1[:],
                                     start=True, stop=False)
                    nc.tensor.matmul(rot[:], rotM2_sb[:], p2[:],
                                     start=False, stop=True)
                    plrA = pa_mv.tile([128, SC], bf16, tag="plrA", name="plrA")
                    nc.vector.tensor_scalar_mul(plrA[:], rot[:], mA_sb[:])
                    nc.sync.dma_start(a2a_in[p][g2, 128:256, :], plrA[:])
                    plrB = pa_mv.tile([128, SC], bf16, tag="plrB", name="plrB")
                    nc.vector.tensor_scalar_mul(plrB[:], rot[:], mB_sb[:])
                    nc.sync.dma_start(a2a_in[p][G + g2, 128:256, :], plrB[:])
                nc.gpsimd.collective_compute(
                    "AllToAll", mybir.AluOpType.bypass,
                    replica_groups=[list(range(NC))],
                    ins=[a2a_in[p].opt()], outs=[a2a_out[p].opt()])

        # ================= Phase C: kv decompress + assembly =================
        persist = top.enter_context(tc.tile_pool(name="persist", bufs=1))
        wpj = []
        for ot in range(H * HD // 128):
            t = persist.tile([128, SC], bf16, tag=f"wpj{ot}", name=f"wpj{ot}")
            nc.sync.dma_start(t[:], P_wproj[ot * 128:(ot + 1) * 128, :])
            wpj.append(t)
        kT = [persist.tile([128, S], bf16, tag=f"kT{h}", name=f"kT{h}") for h in range(HC)]
        v_sb = [persist.tile([128, HC * HD], bf16, tag=f"v{t}", name=f"v{t}") for t in range(NT)]

        with ExitStack() as ctxC:
            pc = ctxC.enter_context(tc.tile_pool(name="pc", bufs=1))
            pc_ps = ctxC.enter_context(
                tc.tile_pool(name="pc_ps", bufs=4, space="PSUM"))

            # A2A payload assembly (gpsimd queue: runs as each A2A lands):
            # batch-A halves land directly in qT/kT; batch-B halves go to
            # temps added in place later (wrong-batch copy is zeros).
            asm = {}
            for p in range(2):
                h0, h1 = 2 * p, 2 * p + 1
                nc.gpsimd.dma_start(
                    qT[h0][0:64, :].rearrange("p (g c) -> p g c", g=G),
                    a2a_out[p][0:G, 0:64, :].rearrange("g p c -> p g c"))
                nc.gpsimd.dma_start(
                    qT[h1][64:128, :].rearrange("p (g c) -> p g c", g=G),
                    a2a_out[p][0:G, 64:128, :].rearrange("g p c -> p g c"))
                tnB = pc.tile([128, S], bf16, tag=f"tnB{p}", name=f"tnB{p}")
                nc.gpsimd.dma_start(
                    tnB[:].rearrange("p (g c) -> p g c", g=G),
                    a2a_out[p][G:2 * G, 0:128, :].rearrange("g p c -> p g c"))
                nc.gpsimd.dma_start(
                    kT[h1][0:64, :].rearrange("p (g c) -> p g c", g=G),
                    a2a_out[p][0:G, 128:192, :].rearrange("g p c -> p g c"))
                nc.gpsimd.dma_start(
                    kT[h0][64:128, :].rearrange("p (g c) -> p g c", g=G),
                    a2a_out[p][0:G, 192:256, :].rearrange("g p c -> p g c"))
                trB = pc.tile([128, S], bf16, tag=f"trB{p}", name=f"trB{p}")
                nc.gpsimd.dma_start(
                    trB[:].rearrange("p (g c) -> p g c", g=G),
                    a2a_out[p][G:2 * G, 128:256, :].rearrange("g p c -> p g c"))
                asm[p] = (tnB, trB)

            def asm_add(p):
                h0, h1 = 2 * p, 2 * p + 1
                tnB, trB = asm[p]
                nc.vector.tensor_add(qT[h0][0:64, :], qT[h0][0:64, :],
                                     tnB[0:64, :])
                nc.vector.tensor_add(qT[h1][64:128, :], qT[h1][64:128, :],
                                     tnB[64:128, :])
                nc.vector.tensor_add(kT[h1][0:64, :], kT[h1][0:64, :],
                                     trB[0:64, :])
                nc.vector.tensor_add(kT[h0][64:128, :], kT[h0][64:128, :],
                                     trB[64:128, :])

            asm_add(0)
            # ---- v (natural layout), two interleaved chains ----
            for st2 in range(NT // 2):
                pv = [pc_ps.tile([128, HC * HD], f32, tag="dec_ps", name="dec_ps")
                      for _ in range(2)]
                for rt in range(KVR // 128):
                    for j in range(2):
                        st = st2 * 2 + j
                        nc.tensor.matmul(
                            pv[j][:], nkvT[rt][:, st * 128:(st + 1) * 128],
                            wdv_sb[rt][:],
                            start=rt == 0, stop=rt == KVR // 128 - 1)
                for j in range(2):
                    nc.scalar.copy(v_sb[st2 * 2 + j][:], pv[j][:])

            # ---- k_nope: head-pair packed, transposed layout ----
            def knope_pair(p):
                psl = [pc_ps.tile([128, 512], f32, tag="dec_ps", name="dec_ps")
                       for _ in range(S // 512)]
                for rt in range(KVR // 128):
                    stat = wdkn_sb[rt][:, p * 128:(p + 1) * 128]
                    for sc4 in range(S // 512):
                        nc.tensor.matmul(
                            psl[sc4][:], stat,
                            nkvT[rt][:, sc4 * 512:(sc4 + 1) * 512],
                            start=rt == 0, stop=rt == KVR // 128 - 1)
                h0, h1 = 2 * p, 2 * p + 1
                for sc4 in range(S // 512):
                    sl = slice(sc4 * 512, (sc4 + 1) * 512)
                    nc.vector.tensor_copy(kT[h0][0:64, sl], psl[sc4][0:64, :])
                    nc.vector.tensor_copy(kT[h1][64:128, sl], psl[sc4][64:128, :])

            knope_pair(0)
            asm_add(1)
            knope_pair(1)

        # ================= Phase D: causal SDPA (4 heads) =================
        with ExitStack() as ctxD:
            pd_mv = ctxD.enter_context(tc.tile_pool(name="pd_mv", bufs=4))
            pd_probs = ctxD.enter_context(tc.tile_pool(name="pd_probs", bufs=6))
            pd_sc = ctxD.enter_context(
                tc.tile_pool(name="pd_sc", bufs=4, space="PSUM"))
            pd_acc = ctxD.enter_context(
                tc.tile_pool(name="pd_acc", bufs=2, space="PSUM"))

            for h in range(HC):
                vcol = slice(h * HD, (h + 1) * HD)
                for w in range(NW):
                    nk = 4 * (w + 1)
                    den = pd_acc.tile([128, 512], f32, tag="den", name="den")
                    att = pd_acc.tile([128, 512], f32, tag="att", name="att")
                    for kt in range(nk):
                        off = max(0, 128 * kt - 512 * w)
                        sq0 = 512 * w + off
                        ssc = pd_sc.tile([128, 512], f32, tag="ssc", name="ssc")
                        nc.tensor.matmul(
                            ssc[:, off:512],
                            kT[h][:, kt * 128:(kt + 1) * 128],
                            qT[h][:, sq0:512 * (w + 1)],
                            start=True, stop=True)
                        if kt >= 4 * w:   # block containing the diagonal
                            nc.vector.tensor_add(
                                ssc[:, off:off + 128],
                                ssc[:, off:off + 128], mask_sb[:])
                        probs = pd_probs.tile([128, 512], bf16, tag="probs", name="probs")
                        nc.scalar.activation(
                            probs[:, off:512], ssc[:, off:512],
                            AF.Exp, scale=SCALE)
                        nc.tensor.matmul(
                            den[:, off:512], ones_sb[:], probs[:, off:512],
                            start=kt == 0, stop=kt == nk - 1)
                        nc.tensor.matmul(
                            att[:, off:512], v_sb[kt][:, vcol],
                            probs[:, off:512],
                            start=kt == 0, stop=kt == nk - 1)
                    rec = pd_mv.tile([128, 512], f32, tag="rec", name="rec")
                    nc.vector.reciprocal(rec[:], den[:])
                    outT = pd_mv.tile([128, 512], bf16, tag="outT", name="outT")
                    nc.vector.tensor_mul(outT[:], att[:], rec[:])
                    nc.sync.dma_start(
                        agh_in[h][:, w * 512:(w + 1) * 512], outT[:])
                nc.gpsimd.collective_compute(
                    "AllGather", mybir.AluOpType.bypass,
                    replica_groups=groups,
                    ins=[agh_in[h].opt()], outs=[agh_out[h].opt()])

        # ===== column-parallel projection (attn-out AGs issued per head) ====
        with ExitStack() as ctxE:
            pe = ctxE.enter_context(tc.tile_pool(name="pe", bufs=1))
            pe_mv = ctxE.enter_context(tc.tile_pool(name="pe_mv", bufs=4))
            pe_ps = ctxE.enter_context(
                tc.tile_pool(name="pe_ps", bufs=5, space="PSUM"))

            aT = [None] * (H * HD // 128)
            for hc in range(HC):          # arrival order: head-chunk 0..3
                for g2 in range(G):
                    ot = 4 * g2 + hc      # global o-tile (= global head)
                    t = pe.tile([128, S], bf16, tag=f"aT{ot}", name=f"aT{ot}")
                    nc.sync.dma_start(t[:], agh_out[hc][g2, :, :])
                    aT[ot] = t
            order = [(hc, g2) for hc in range(HC) for g2 in range(G)]
            for st4 in range(NT // 2):
                pp = [pe_ps.tile([128, SC], f32, tag="proj_ps", name="proj_ps")
                      for _ in range(2)]
                for i, (hc, g2) in enumerate(order):
                    ot = 4 * g2 + hc
                    for j in range(2):
                        st2 = st4 * 2 + j
                        nc.tensor.matmul(
                            pp[j][:], aT[ot][:, st2 * 128:(st2 + 1) * 128],
                            wpj[ot][:],
                            start=i == 0, stop=i == H * HD // 128 - 1)
                for j in range(2):
                    st2 = st4 * 2 + j
                    o_sb = pe_mv.tile([128, SC], f32, tag="o_sb", name="o_sb")
                    nc.scalar.copy(o_sb[:], pp[j][:])
                    nc.sync.dma_start(
                        P_out[st2 * 128:(st2 + 1) * 128, :], o_sb[:])

    nc.compile()
    return nc


def _get_nc():
    if "nc" not in _cached:
        _cached["nc"] = _build()
    return _cached["nc"]


def _prep_inputs(inputs):
    x = np.asarray(inputs["x"], np.float32)
    fc = np.asarray(inputs["freqs_cos"], np.float32)   # [S, 32]
    fs = np.asarray(inputs["freqs_sin"], np.float32)
    w_cq = np.asarray(inputs["w_cq"], np.float32)
    w_dq_nope = np.asarray(inputs["w_dq_nope"], np.float32)
    w_dq_rope = np.asarray(inputs["w_dq_rope"], np.float32)
    w_ckv = np.asarray(inputs["w_ckv"], np.float32)
    w_dk_nope = np.asarray(inputs["w_dk_nope"], np.float32)
    w_dv = np.asarray(inputs["w_dv"], np.float32)
    w_krope = np.asarray(inputs["w_krope"], np.float32)
    w_proj = np.asarray(inputs["w_proj"], np.float32)
    qw = np.asarray(inputs["q_norm_w"], np.float32)
    kvw = np.asarray(inputs["kv_norm_w"], np.float32)

    perm = np.concatenate([np.arange(0, RD, 2), np.arange(1, RD, 2)])

    wlat = np.concatenate(
        [w_cq.T, w_ckv.T, w_krope[perm, :].T], axis=1).astype(BF)  # [D, LATW]
    wdqn = (w_dq_nope * qw[None, :]).reshape(H, ND, QR)
    wdqr = (w_dq_rope * qw[None, :]).reshape(H, RD, QR)[:, perm, :]
    wdkn = (w_dk_nope * kvw[None, :])
    wdv = (w_dv * kvw[None, :])
    wprojT = np.ascontiguousarray(w_proj.T).astype(BF)

    # all-heads decompress weights, column blocks ordered (pair, dest):
    # block (p*G+g2) covers heads (4*g2+2p, 4*g2+2p+1)
    head_order = []
    for p in range(2):
        for g2 in range(G):
            head_order += [4 * g2 + 2 * p, 4 * g2 + 2 * p + 1]
    wdqnA = np.ascontiguousarray(
        wdqn[head_order].reshape(H * ND, QR).T).astype(BF)
    wdqrA = np.ascontiguousarray(
        wdqr[head_order].reshape(H * RD, QR).T).astype(BF)

    # banded transposed rope tables [128, S]
    tabA = np.empty((128, S), np.float32)
    tabB = np.empty((128, S), np.float32)
    for band in range(4):
        rows = slice(band * 32, (band + 1) * 32)
        if band % 2 == 0:
            tabA[rows] = fc.T
            tabB[rows] = fs.T
        else:
            tabA[rows] = fs.T
            tabB[rows] = fc.T
    tabA = tabA.astype(BF)
    tabB = tabB.astype(BF)

    # band-combine selectors: rot = rotM1.T @ (x*tabA) + rotM2.T @ (x*tabB)
    # output 64-blocks are SWAPPED vs input blocks so that PSUM rows land on
    # the same base partitions as their kT destinations (even-head rope at
    # partitions 64:128, odd-head rope at 0:64).
    rotM1 = np.zeros((128, 128), np.float32)
    rotM2 = np.zeros((128, 128), np.float32)
    for b in range(2):
        for i in range(32):
            ei, oi = b * 64 + i, b * 64 + 32 + i            # input rows
            eo, oo = (1 - b) * 64 + i, (1 - b) * 64 + 32 + i  # output rows
            rotM1[ei, eo] = 1.0
            rotM1[oi, eo] = -1.0
            rotM2[ei, oo] = 1.0
            rotM2[oi, oo] = 1.0
    rotM1 = rotM1.astype(BF)
    rotM2 = rotM2.astype(BF)

    maskT = np.zeros((128, 128), np.float32)
    il, jl = np.tril_indices(128, -1)   # sq < sk  -> masked
    maskT[il, jl] = NEG

    in_maps = []
    for c in range(NC):
        b, g = divmod(c, G)
        hsl = slice(g * HC, (g + 1) * HC)
        ssl = slice(g * SC, (g + 1) * SC)
        xT_c = np.ascontiguousarray(x[b].T[:, ssl]).astype(BF)
        wdkn_c = np.ascontiguousarray(
            wdkn.reshape(H, ND, KVR)[hsl].reshape(HC * ND, KVR).T).astype(BF)
        wdv_c = np.ascontiguousarray(
            wdv.reshape(H, HD, KVR)[hsl].reshape(HC * HD, KVR).T).astype(BF)
        wproj_c = np.ascontiguousarray(wprojT[:, ssl])
        in_maps.append({
            "xT": xT_c,
            "wlat": wlat,
            "wdqn": wdqnA,
            "wdqr": wdqrA,
            "wdkn": wdkn_c,
            "wdv": wdv_c,
            "wproj": wproj_c,
            "tabA": np.ascontiguousarray(tabA[:, ssl]),
            "tabB": np.ascontiguousarray(tabB[:, ssl]),
            "tabKA": np.ascontiguousarray(
                np.concatenate([fc[ssl].T, fs[ssl].T], axis=0)).astype(BF),
            "tabKB": np.ascontiguousarray(
                np.concatenate([fs[ssl].T, fc[ssl].T], axis=0)).astype(BF),
            "rotM1": rotM1,
            "rotM2": rotM2,
            "mA": np.full((128, 1), 1.0 - b, np.float32),
            "mB": np.full((128, 1), float(b), np.float32),
            "maskT": maskT,
        })
    return in_maps


def _assemble(results):
    out = np.zeros((B, S, H * HD), np.float32)
    for c in range(NC):
        b, g = divmod(c, G)
        out[b, :, g * SC:(g + 1) * SC] = results[c]["out"]
    return out


def kernel(**inputs) -> np.ndarray:
    from concourse.bass_utils import run_bass_kernel_spmd
    nc = _get_nc()
    in_maps = _prep_inputs(inputs)
    res = run_bass_kernel_spmd(nc, in_maps, core_ids=list(range(NC)))
    return _assemble(res.results)


# revision 58
# speedup vs baseline: 1.6652x; 1.0421x over previous
"""Multi-Head Latent Attention (MLA) Trainium2 kernel, 8-core SPMD.

Sharding: core c -> batch b = c // 4, head-group g = c % 4 (4 heads each).

v3 structure:
 - Phase A produces TRANSPOSED latents directly on the PE (stationary = wlat
   column block, moving = xT chunk); kv blocks first so the small gkv
   AllGather (normalized kv latent + rotated shared krope) fires early.
 - The q latent never leaves the core: each core decompresses q_nope and
   rotated q_rope for ALL 16 heads on its own S chunk (the transposed latent
   tiles feed the decompress directly as moving operands), applies its local
   rmsnorm scale during PSUM eviction, and exchanges per-head payloads with
   two pipelined AllToAlls (head pair 0 first, so SDPA on heads 0/1 starts
   while pair 1 is still in flight).
 - Rope rotation runs in the transposed layout: banded cos/sin tables
   (vector muls) + a +-1 band-combine matmul pair on the PE.
 - SDPA computes scores transposed ([sk, sq]); denominators via ones-matmul.
 - Output projection is column-parallel via per-core w_proj column slices.
"""

import sys

for _p in ("/opt/trn_rl_repo", "/opt/pypackages"):
    if _p not in sys.path:
        sys.path.append(_p)

import numpy as np
import ml_dtypes

B, S, D = 2, 2048, 2048
H, HD, RD, ND = 16, 128, 64, 64
QR, KVR = 1536, 512
EPS = 1e-6
G = 4            # cores per batch group
NC = 8
HC = H // G      # heads per core
SC = S // G      # latent-path S chunk per core
NT = S // 128    # 16 s-tiles
NW = S // 512    # 4  sq windows
LATW = QR + KVR + RD   # 2112 = packed cq|ckv|krope width
SCALE = 1.0 / float(np.sqrt(HD))
NEG = -30000.0   # additive mask; * SCALE stays << exp underflow

BF = ml_dtypes.bfloat16

_cached = {}


def _build():
    import concourse.bass as bass
    import concourse.bass_isa as bass_isa
    import concourse.mybir as mybir
    import concourse.tile as tile
    from concourse import bacc
    from contextlib import ExitStack

    f32 = mybir.dt.float32
    bf16 = mybir.dt.bfloat16
    AF = mybir.ActivationFunctionType

    nc = bacc.Bacc()

    # ---- parameters (host-prepped) ----
    P_xT = nc.declare_dram_parameter("xT", [D, SC], bf16, isOutput=False)
    P_wlat = nc.declare_dram_parameter("wlat", [D, LATW], bf16, isOutput=False)
    # all-heads decompress weights, (pair, dest-core) packed columns
    P_wdqn = nc.declare_dram_parameter("wdqn", [QR, H * ND], bf16, isOutput=False)
    P_wdqr = nc.declare_dram_parameter("wdqr", [QR, H * RD], bf16, isOutput=False)
    P_wdkn = nc.declare_dram_parameter("wdkn", [KVR, HC * ND], bf16, isOutput=False)
    P_wdv = nc.declare_dram_parameter("wdv", [KVR, HC * HD], bf16, isOutput=False)
    P_wproj = nc.declare_dram_parameter("wproj", [H * HD, SC], bf16, isOutput=False)
    # banded transposed rope tables for OWN chunk [128, SC] (see _prep_inputs)
    P_tabA = nc.declare_dram_parameter("tabA", [128, SC], bf16, isOutput=False)
    P_tabB = nc.declare_dram_parameter("tabB", [128, SC], bf16, isOutput=False)
    # krope rotation tables for own chunk (transposed, banded): [64, SC]
    P_tabKA = nc.declare_dram_parameter("tabKA", [64, SC], bf16, isOutput=False)
    P_tabKB = nc.declare_dram_parameter("tabKB", [64, SC], bf16, isOutput=False)
    # rope band-combine selector matrices (+-1), see _prep_inputs
    P_rotM1 = nc.declare_dram_parameter("rotM1", [128, 128], bf16, isOutput=False)
    P_rotM2 = nc.declare_dram_parameter("rotM2", [128, 128], bf16, isOutput=False)
    # batch masks: mA = 1 if this core is in batch 0 else 0; mB = 1 - mA
    P_mA = nc.declare_dram_parameter("mA", [128, 1], f32, isOutput=False)
    P_mB = nc.declare_dram_parameter("mB", [128, 1], f32, isOutput=False)
    P_mask = nc.declare_dram_parameter("maskT", [128, 128], f32, isOutput=False)
    P_out = nc.declare_dram_parameter("out", [S, SC], f32, isOutput=True)

    groups = [[0, 1, 2, 3], [4, 5, 6, 7]]

    with ExitStack() as top:
        tc = top.enter_context(tile.TileContext(nc))

        dram = top.enter_context(tc.tile_pool(name="dram", bufs=1, space="DRAM"))
        KVW = KVR + RD   # 576
        gkv_in = dram.tile([KVW, SC], bf16, tag="gkv_in", name="gkv_in")
        gkv_out = dram.tile([G, KVW, SC], bf16, tag="gkv_out", name="gkv_out")
        # AllToAll payloads over ALL 8 cores: per head-pair, per dest core:
        # [nope(128); rope(128)].  Each sender writes its payload to BOTH
        # batch-halves' blocks, masked by mA/mB (wrong-batch copy is zeros);
        # receivers sum block g2 and block 4+g2.  Keeps addressing static.
        a2a_in = [dram.tile([NC, 256, SC], bf16, tag=f"a2a_in{p}", name=f"a2a_in{p}")
                  for p in range(2)]
        a2a_out = [dram.tile([NC, 256, SC], bf16, tag=f"a2a_out{p}", name=f"a2a_out{p}")
                   for p in range(2)]
        agh_in = [dram.tile([HD, S], bf16, tag=f"agh_in{h}", name=f"agh_in{h}")
                  for h in range(HC)]
        agh_out = [dram.tile([G, HD, S], bf16, tag=f"agh_out{h}", name=f"agh_out{h}")
                   for h in range(HC)]

        const = top.enter_context(tc.tile_pool(name="const", bufs=1))
        ones_sb = const.tile([128, 128], bf16, tag="ones", name="ones")
        nc.vector.memset(ones_sb[:], 1.0)
        onesf = const.tile([1, 128], f32, tag="onesf", name="onesf")
        nc.vector.memset(onesf[:], 1.0)
        mask_sb = const.tile([128, 128], f32, tag="mask", name="mask")
        nc.sync.dma_start(mask_sb[:], P_mask[:])
        eps_sb = const.tile([128, 1], f32, tag="eps", name="eps")
        nc.vector.memset(eps_sb[:], EPS)
        rotM1_sb = const.tile([128, 128], bf16, tag="rotM1", name="rotM1")
        nc.sync.dma_start(rotM1_sb[:], P_rotM1[:])
        rotM2_sb = const.tile([128, 128], bf16, tag="rotM2", name="rotM2")
        nc.sync.dma_start(rotM2_sb[:], P_rotM2[:])
        mA_sb = const.tile([128, 1], f32, tag="mA", name="mA")
        nc.sync.dma_start(mA_sb[:], P_mA[:])
        mB_sb = const.tile([128, 1], f32, tag="mB", name="mB")
        nc.sync.dma_start(mB_sb[:], P_mB[:])

        # tiles that must survive phase A into phase C/D
        pk = top.enter_context(tc.tile_pool(name="pk", bufs=1))
        nkvT = [pk.tile([128, S], bf16, tag=f"nkvT{rt}", name=f"nkvT{rt}")
                for rt in range(KVR // 128)]
        qT = [pk.tile([128, S], bf16, tag=f"qT{h}", name=f"qT{h}")
              for h in range(HC)]
        wdkn_sb = [pk.tile([128, HC * ND], bf16, tag=f"wdkn{rt}", name=f"wdkn{rt}")
                   for rt in range(KVR // 128)]
        wdv_sb = [pk.tile([128, HC * HD], bf16, tag=f"wdv{rt}", name=f"wdv{rt}")
                  for rt in range(KVR // 128)]

        # ===== Phase A: transposed latents + local q decompress + A2A =====
        with ExitStack() as ctxA:
            pa = ctxA.enter_context(tc.tile_pool(name="pa", bufs=1))
            pa_mv = ctxA.enter_context(tc.tile_pool(name="pa_mv", bufs=3))
            pa_ps = ctxA.enter_context(
                tc.tile_pool(name="pa_ps", bufs=4, space="PSUM"))
            pa_ss = ctxA.enter_context(
                tc.tile_pool(name="pa_ss", bufs=1, space="PSUM"))

            pa_w = ctxA.enter_context(tc.tile_pool(name="pa_w", bufs=2))

            xT_sb = []
            for dt_ in range(D // 128):
                xt = pa.tile([128, SC], bf16, tag=f"xT{dt_}", name=f"xT{dt_}")
                nc.sync.dma_start(xt[:], P_xT[dt_ * 128:(dt_ + 1) * 128, :])
                xT_sb.append(xt)
            for rt in range(KVR // 128):
                nc.sync.dma_start(wdkn_sb[rt][:],
                                  P_wdkn[rt * 128:(rt + 1) * 128, :])
                nc.sync.dma_start(wdv_sb[rt][:],
                                  P_wdv[rt * 128:(rt + 1) * 128, :])

            def wl_chunk(col0, ncols):
                # stream a [D, ncols] column chunk of wlat into a ring
                tiles = []
                for dt_ in range(D // 128):
                    t = pa_w.tile([128, 256], bf16, tag=f"wlc{dt_}",
                                  name=f"wlc{dt_}")
                    nc.sync.dma_start(
                        t[:, 0:ncols],
                        P_wlat[dt_ * 128:(dt_ + 1) * 128, col0:col0 + ncols])
                    tiles.append(t)
                return tiles

            tabKA_sb = pa.tile([64, SC], bf16, tag="tabKA", name="tabKA")
            nc.sync.dma_start(tabKA_sb[:], P_tabKA[:])
            tabKB_sb = pa.tile([64, SC], bf16, tag="tabKB", name="tabKB")
            nc.sync.dma_start(tabKB_sb[:], P_tabKB[:])
            tabA_sb = pa.tile([128, SC], bf16, tag="tabA", name="tabA")
            nc.sync.dma_start(tabA_sb[:], P_tabA[:])
            tabB_sb = pa.tile([128, SC], bf16, tag="tabB", name="tabB")
            nc.sync.dma_start(tabB_sb[:], P_tabB[:])

            def lat_pair(wl, cols, psums):
                # cols: list of (local col0, ncols) matching psums
                for dt_ in range(D // 128):
                    for (col0, ncols), ps in zip(cols, psums):
                        nc.tensor.matmul(
                            ps[0:ncols, :],
                            wl[dt_][:, col0:col0 + ncols],
                            xT_sb[dt_][:],
                            start=dt_ == 0, stop=dt_ == D // 128 - 1)

            # ---- kv blocks (2 pairs) + sumsq ----
            sskv = pa_ss.tile([1, SC], f32, tag="sskv", name="sskv")
            kvraw = []
            kvsq = []
            for pr in range(2):
                wl = wl_chunk(1536 + pr * 256, 256)
                psA = pa_ps.tile([128, SC], f32, tag="lat_ps", name="lat_ps")
                psB = pa_ps.tile([128, SC], f32, tag="lat_ps", name="lat_ps")
                lat_pair(wl, [(0, 128), (128, 128)], [psA, psB])
                for j, ps in enumerate((psA, psB)):
                    rb = pr * 2 + j
                    raw = pa.tile([128, SC], bf16, tag=f"kvraw{rb}", name=f"kvraw{rb}")
                    nc.scalar.copy(raw[:], ps[:])
                    kvraw.append(raw)
                    sq = pa.tile([128, SC], bf16, tag=f"sqkv{rb}", name=f"sqkv{rb}")
                    nc.scalar.activation(sq[:], ps[:], AF.Square)
                    kvsq.append(sq)
            # ---- krope block: rotate (no norm) ----
            wl = wl_chunk(2048, 64)
            pkr = pa_ps.tile([128, SC], f32, tag="lat_ps", name="lat_ps")
            lat_pair(wl, [(0, 64)], [pkr])
            for rb in range(4):
                nc.tensor.matmul(sskv[:], ones_sb[:, 0:1], kvsq[rb][:],
                                 start=rb == 0, stop=rb == 3)
            kr1 = pa.tile([64, SC], bf16, tag="kr1", name="kr1")
            kr2 = pa.tile([64, SC], bf16, tag="kr2", name="kr2")
            nc.vector.tensor_mul(kr1[:], pkr[0:64, :], tabKA_sb[:])
            nc.vector.tensor_mul(kr2[:], pkr[0:64, :], tabKB_sb[:])
            rotp = pa_ss.tile([128, SC], f32, tag="rot", name="rot")
            nc.tensor.matmul(rotp[0:64, :], rotM1_sb[0:64, 64:128], kr1[:],
                             start=True, stop=False)
            nc.tensor.matmul(rotp[0:64, :], rotM2_sb[0:64, 64:128], kr2[:],
                             start=False, stop=True)
            krot = pa.tile([64, SC], bf16, tag="krot", name="krot")
            nc.scalar.copy(krot[:], rotp[0:64, :])
            nc.sync.dma_start(gkv_in[KVR:KVW, :], krot[:])

            # ---- kv norm: rsqrt(mean sq + eps), broadcast, scale, ship ----
            skv = pa.tile([1, SC], f32, tag="skv", name="skv")
            nc.scalar.activation(skv[:], sskv[:], AF.Sqrt,
                                 bias=eps_sb[0:1, :], scale=1.0 / KVR)
            rkv = pa.tile([1, SC], f32, tag="rkv", name="rkv")
            nc.vector.reciprocal(rkv[:], skv[:])
            rkvb = pa.tile([128, SC], f32, tag="rkvb", name="rkvb")
            nc.gpsimd.partition_broadcast(rkvb[:], rkv[:], channels=128)
            for rb in range(4):
                kvn = pa_mv.tile([128, SC], bf16, tag="kvn", name="kvn")
                nc.vector.tensor_mul(kvn[:], kvraw[rb][:], rkvb[:])
                nc.sync.dma_start(
                    gkv_in[rb * 128:(rb + 1) * 128, :], kvn[:])

            nc.gpsimd.collective_compute(
                "AllGather", mybir.AluOpType.bypass,
                replica_groups=groups,
                ins=[gkv_in.opt()], outs=[gkv_out.opt()])

            # kv-latent + shared-krope loads: issued on the gpsimd queue right
            # behind the AG trigger so they run the moment the AG lands.
            for rt in range(KVR // 128):
                nc.gpsimd.dma_start(
                    nkvT[rt][:].rearrange("p (g c) -> p g c", g=G),
                    gkv_out[:, rt * 128:(rt + 1) * 128, :].rearrange(
                        "g p c -> p g c"))
            for h in range(HC):
                roff = 64 if h % 2 == 0 else 0   # even: [nope|rope], odd: [rope|nope]
                nc.gpsimd.dma_start(
                    qT[h][roff:roff + 64, :].rearrange(
                        "p (g c) -> p g c", g=G),
                    gkv_out[:, KVR:KVW, :].rearrange("g p c -> p g c"))

            # ---- q blocks (6 pairs), raw latents stay in SBUF ----
            # (q decompress weights stream in alongside, 2 tiles per pair)
            wdqnA = [pa.tile([128, H * ND], bf16, tag=f"wdqnA{rt}",
                             name=f"wdqnA{rt}") for rt in range(QR // 128)]
            wdqrA = [pa.tile([128, H * RD], bf16, tag=f"wdqrA{rt}",
                             name=f"wdqrA{rt}") for rt in range(QR // 128)]
            ssq = pa_ss.tile([1, SC], f32, tag="ssq", name="ssq")
            latq = []
            pend = []   # delayed sumsq ones-matmuls: (sq_tile, rb)
            for pr in range(6):
                wl = wl_chunk(pr * 256, 256)
                for rt in (2 * pr, 2 * pr + 1):
                    nc.sync.dma_start(wdqnA[rt][:],
                                      P_wdqn[rt * 128:(rt + 1) * 128, :])
                    nc.sync.dma_start(wdqrA[rt][:],
                                      P_wdqr[rt * 128:(rt + 1) * 128, :])
                psA = pa_ps.tile([128, SC], f32, tag="lat_ps", name="lat_ps")
                psB = pa_ps.tile([128, SC], f32, tag="lat_ps", name="lat_ps")
                lat_pair(wl, [(0, 128), (128, 128)], [psA, psB])
                while pend:
                    t, prb = pend.pop(0)
                    nc.tensor.matmul(ssq[:], ones_sb[:, 0:1], t[:],
                                     start=prb == 0, stop=False)
                for j, ps in enumerate((psA, psB)):
                    rb = pr * 2 + j
                    raw = pa.tile([128, SC], bf16, tag=f"latq{rb}", name=f"latq{rb}")
                    nc.scalar.copy(raw[:], ps[:])
                    latq.append(raw)
                    sq = pa_mv.tile([128, SC], bf16, tag="sq", name="sq")
                    nc.scalar.activation(sq[:], ps[:], AF.Square)
                    pend.append((sq, rb))
            for t, prb in pend:
                nc.tensor.matmul(ssq[:], ones_sb[:, 0:1], t[:],
                                 start=False, stop=prb == 11)
            pend = []

            # ---- local q rmsnorm scale ----
            sqr = pa.tile([1, SC], f32, tag="sqr", name="sqr")
            nc.scalar.activation(sqr[:], ssq[:], AF.Sqrt,
                                 bias=eps_sb[0:1, :], scale=1.0 / QR)
            rqr = pa.tile([1, SC], f32, tag="rqr", name="rqr")
            nc.vector.reciprocal(rqr[:], sqr[:])
            rqb = pa.tile([128, SC], f32, tag="rqb", name="rqb")
            nc.gpsimd.partition_broadcast(rqb[:], rqr[:], channels=128)
            tabArq = pa.tile([128, SC], bf16, tag="tabArq", name="tabArq")
            nc.vector.tensor_mul(tabArq[:], tabA_sb[:], rqb[:])
            tabBrq = pa.tile([128, SC], bf16, tag="tabBrq", name="tabBrq")
            nc.vector.tensor_mul(tabBrq[:], tabB_sb[:], rqb[:])
            rqbA = pa.tile([128, SC], bf16, tag="rqbA", name="rqbA")
            nc.vector.tensor_scalar_mul(rqbA[:], rqb[:], mA_sb[:])
            rqbB = pa.tile([128, SC], bf16, tag="rqbB", name="rqbB")
            nc.vector.tensor_scalar_mul(rqbB[:], rqb[:], mB_sb[:])

            # ---- local q decompress (all 16 heads, own chunk) + A2A ----
            for p in range(2):          # head pair within each dest
                for g2 in range(G):     # dest core, one at a time
                    psn = pa_ps.tile([128, SC], f32, tag="lat_ps", name="lat_ps")
                    psr = pa_ps.tile([128, SC], f32, tag="lat_ps", name="lat_ps")
                    col = (p * G + g2) * 128
                    for rt in range(QR // 128):
                        nc.tensor.matmul(
                            psn[:], wdqnA[rt][:, col:col + 128], latq[rt][:],
                            start=rt == 0, stop=rt == QR // 128 - 1)
                        nc.tensor.matmul(
                            psr[:], wdqrA[rt][:, col:col + 128], latq[rt][:],
                            start=rt == 0, stop=rt == QR // 128 - 1)
                    plnA = pa_mv.tile([128, SC], bf16, tag="plnA", name="plnA")
                    nc.vector.tensor_mul(plnA[0:64, :], psn[0:64, :],
                                         rqbA[0:64, :])
                    nc.vector.tensor_mul(plnA[64:128, :], psn[64:128, :],
                                         rqbA[64:128, :])
                    nc.sync.dma_start(a2a_in[p][g2, 0:128, :], plnA[:])
                    plnB = pa_mv.tile([128, SC], bf16, tag="plnB", name="plnB")
                    nc.vector.tensor_mul(plnB[0:64, :], psn[0:64, :],
                                         rqbB[0:64, :])
                    nc.vector.tensor_mul(plnB[64:128, :], psn[64:128, :],
                                         rqbB[64:128, :])
                    nc.sync.dma_start(a2a_in[p][G + g2, 0:128, :], plnB[:])
                    p1 = pa_mv.tile([128, SC], bf16, tag="p1", name="p1")
                    p2 = pa_mv.tile([128, SC], bf16, tag="p2", name="p2")
                    nc.vector.tensor_mul(p1[:], psr[:], tabArq[:])
                    nc.vector.tensor_mul(p2[:], psr[:], tabBrq[:])
                    rot = pa_ss.tile([128, SC], f32, tag="rot", name="rot")
                    nc.tensor.matmul(rot[:], rotM1_sb[:], p1[:],
                                     start=True, stop=False)
                    nc.tensor.matmul(rot[:], rotM2_sb[:], p2[:],
                                     start=False, stop=True)
                    plrA = pa_mv.tile([128, SC], bf16, tag="plrA", name="plrA")
                    nc.vector.tensor_scalar_mul(plrA[:], rot[:], mA_sb[:])
                    nc.sync.dma_start(a2a_in[p][g2, 128:256, :], plrA[:])
                    plrB = pa_mv.tile([128, SC], bf16, tag="plrB", name="plrB")
                    nc.vector.tensor_scalar_mul(plrB[:], rot[:], mB_sb[:])
                    nc.sync.dma_start(a2a_in[p][G + g2, 128:256, :], plrB[:])
                nc.gpsimd.collective_compute(
                    "AllToAll", mybir.AluOpType.bypass,
                    replica_groups=[list(range(NC))],
                    ins=[a2a_in[p].opt()], outs=[a2a_out[p].opt()])

        # ================= Phase C: kv decompress + assembly =================
        persist = top.enter_context(tc.tile_pool(name="persist", bufs=1))
        wpj = []
        for ot in range(H * HD // 128):
            t = persist.tile([128, SC], bf16, tag=f"wpj{ot}", name=f"wpj{ot}")
            nc.sync.dma_start(t[:], P_wproj[ot * 128:(ot + 1) * 128, :])
            wpj.append(t)
        kT = [persist.tile([128, S], bf16, tag=f"kT{h}", name=f"kT{h}") for h in range(HC)]
        v_sb = [persist.tile([128, HC * HD], bf16, tag=f"v{t}", name=f"v{t}") for t in range(NT)]

        with ExitStack() as ctxC:
            pc = ctxC.enter_context(tc.tile_pool(name="pc", bufs=1))
            pc_ps = ctxC.enter_context(
                tc.tile_pool(name="pc_ps", bufs=4, space="PSUM"))

            # A2A payload assembly (gpsimd queue: runs as each A2A lands):
            # batch-A halves land directly in qT/kT; batch-B halves go to
            # temps added in place later (wrong-batch copy is zeros).
            asm = {}
            for p in range(2):
                h0, h1 = 2 * p, 2 * p + 1
                nc.gpsimd.dma_start(
                    qT[h0][0:64, :].rearrange("p (g c) -> p g c", g=G),
                    a2a_out[p][0:G, 0:64, :].rearrange("g p c -> p g c"))
                nc.gpsimd.dma_start(
                    qT[h1][64:128, :].rearrange("p (g c) -> p g c", g=G),
                    a2a_out[p][0:G, 64:128, :].rearrange("g p c -> p g c"))
                tnB = pc.tile([128, S], bf16, tag=f"tnB{p}", name=f"tnB{p}")
                nc.gpsimd.dma_start(
                    tnB[:].rearrange("p (g c) -> p g c", g=G),
                    a2a_out[p][G:2 * G, 0:128, :].rearrange("g p c -> p g c"))
                nc.gpsimd.dma_start(
                    kT[h1][0:64, :].rearrange("p (g c) -> p g c", g=G),
                    a2a_out[p][0:G, 128:192, :].rearrange("g p c -> p g c"))
                nc.gpsimd.dma_start(
                    kT[h0][64:128, :].rearrange("p (g c) -> p g c", g=G),
                    a2a_out[p][0:G, 192:256, :].rearrange("g p c -> p g c"))
                trB = pc.tile([128, S], bf16, tag=f"trB{p}", name=f"trB{p}")
                nc.gpsimd.dma_start(
                    trB[:].rearrange("p (g c) -> p g c", g=G),
                    a2a_out[p][G:2 * G, 128:256, :].rearrange("g p c -> p g c"))
                asm[p] = (tnB, trB)

            def asm_add(p):
                h0, h1 = 2 * p, 2 * p + 1
                tnB, trB = asm[p]
                nc.vector.tensor_add(qT[h0][0:64, :], qT[h0][0:64, :],
                                     tnB[0:64, :])
                nc.vector.tensor_add(qT[h1][64:128, :], qT[h1][64:128, :],
                                     tnB[64:128, :])
                nc.vector.tensor_add(kT[h1][0:64, :], kT[h1][0:64, :],
                                     trB[0:64, :])
                nc.vector.tensor_add(kT[h0][64:128, :], kT[h0][64:128, :],
                                     trB[64:128, :])

            def knope_pair(p):
                psl = [pc_ps.tile([128, 512], f32, tag="dec_ps", name="dec_ps")
                       for _ in range(S // 512)]
                for rt in range(KVR // 128):
                    stat = wdkn_sb[rt][:, p * 128:(p + 1) * 128]
                    for sc4 in range(S // 512):
                        nc.tensor.matmul(
                            psl[sc4][:], stat,
                            nkvT[rt][:, sc4 * 512:(sc4 + 1) * 512],
                            start=rt == 0, stop=rt == KVR // 128 - 1)
                h0, h1 = 2 * p, 2 * p + 1
                for sc4 in range(S // 512):
                    sl = slice(sc4 * 512, (sc4 + 1) * 512)
                    nc.vector.tensor_copy(kT[h0][0:64, sl], psl[sc4][0:64, :])
                    nc.vector.tensor_copy(kT[h1][64:128, sl], psl[sc4][64:128, :])

            asm_add(0)
            knope_pair(0)
            # ---- v (natural layout), two interleaved chains ----
            for st2 in range(NT // 2):
                pv = [pc_ps.tile([128, HC * HD], f32, tag="dec_ps", name="dec_ps")
                      for _ in range(2)]
                for rt in range(KVR // 128):
                    for j in range(2):
                        st = st2 * 2 + j
                        nc.tensor.matmul(
                            pv[j][:], nkvT[rt][:, st * 128:(st + 1) * 128],
                            wdv_sb[rt][:],
                            start=rt == 0, stop=rt == KVR // 128 - 1)
                for j in range(2):
                    nc.scalar.copy(v_sb[st2 * 2 + j][:], pv[j][:])

            # ---- k_nope: head-pair packed, transposed layout ----
            asm_add(1)
            knope_pair(1)

        # ================= Phase D: causal SDPA (4 heads) =================
        with ExitStack() as ctxD:
            pd_mv = ctxD.enter_context(tc.tile_pool(name="pd_mv", bufs=4))
            pd_probs = ctxD.enter_context(tc.tile_pool(name="pd_probs", bufs=6))
            pd_sc = ctxD.enter_context(
                tc.tile_pool(name="pd_sc", bufs=4, space="PSUM"))
            pd_acc = ctxD.enter_context(
                tc.tile_pool(name="pd_acc", bufs=2, space="PSUM"))

            for h in range(HC):
                vcol = slice(h * HD, (h + 1) * HD)
                for w in range(NW):
                    nk = 4 * (w + 1)
                    den = pd_acc.tile([128, 512], f32, tag="den", name="den")
                    att = pd_acc.tile([128, 512], f32, tag="att", name="att")
                    for kt in range(nk):
                        off = max(0, 128 * kt - 512 * w)
                        sq0 = 512 * w + off
                        ssc = pd_sc.tile([128, 512], f32, tag="ssc", name="ssc")
                        nc.tensor.matmul(
                            ssc[:, off:512],
                            kT[h][:, kt * 128:(kt + 1) * 128],
                            qT[h][:, sq0:512 * (w + 1)],
                            start=True, stop=True)
                        if kt >= 4 * w:   # block containing the diagonal
                            nc.vector.tensor_add(
                                ssc[:, off:off + 128],
                                ssc[:, off:off + 128], mask_sb[:])
                        probs = pd_probs.tile([128, 512], bf16, tag="probs", name="probs")
                        nc.scalar.activation(
                            probs[:, off:512], ssc[:, off:512],
                            AF.Exp, scale=SCALE)
                        nc.tensor.matmul(
                            den[:, off:512], ones_sb[:], probs[:, off:512],
                            start=kt == 0, stop=kt == nk - 1)
                        nc.tensor.matmul(
                            att[:, off:512], v_sb[kt][:, vcol],
                            probs[:, off:512],
                            start=kt == 0, stop=kt == nk - 1)
                    rec = pd_mv.tile([128, 512], f32, tag="rec", name="rec")
                    nc.vector.reciprocal(rec[:], den[:])
                    outT = pd_mv.tile([128, 512], bf16, tag="outT", name="outT")
                    nc.vector.tensor_mul(outT[:], att[:], rec[:])
                    nc.sync.dma_start(
                        agh_in[h][:, w * 512:(w + 1) * 512], outT[:])
                nc.gpsimd.collective_compute(
                    "AllGather", mybir.AluOpType.bypass,
                    replica_groups=groups,
                    ins=[agh_in[h].opt()], outs=[agh_out[h].opt()])

        # ===== column-parallel projection (attn-out AGs issued per head) ====
        with ExitStack() as ctxE:
            pe = ctxE.enter_context(tc.tile_pool(name="pe", bufs=1))
            pe_mv = ctxE.enter_context(tc.tile_pool(name="pe_mv", bufs=4))
            pe_ps = ctxE.enter_context(
                tc.tile_pool(name="pe_ps", bufs=5, space="PSUM"))

            # arrival-order accumulation: as each head-chunk's AllGather
            # lands, fold its 4 o-tiles into SBUF f32 accumulators, so only
            # the LAST chunk's matmuls remain after the final AG.
            acc = [pe.tile([128, SC], f32, tag=f"acc{st}", name=f"acc{st}")
                   for st in range(NT)]
            aT = [None] * (H * HD // 128)
            for hc in range(HC):          # arrival order: head-chunk 0..3
                for g2 in range(G):
                    ot = 4 * g2 + hc      # global o-tile (= global head)
                    t = pe.tile([128, S], bf16, tag=f"aT{ot}", name=f"aT{ot}")
                    nc.sync.dma_start(t[:], agh_out[hc][g2, :, :])
                    aT[ot] = t
                for st4 in range(NT // 2):
                    pp = [pe_ps.tile([128, SC], f32, tag="proj_ps",
                                     name="proj_ps") for _ in range(2)]
                    for g2 in range(G):
                        ot = 4 * g2 + hc
                        for j in range(2):
                            st2 = st4 * 2 + j
                            nc.tensor.matmul(
                                pp[j][:],
                                aT[ot][:, st2 * 128:(st2 + 1) * 128],
                                wpj[ot][:],
                                start=g2 == 0, stop=g2 == G - 1)
                    for j in range(2):
                        st2 = st4 * 2 + j
                        if hc == 0:
                            nc.vector.tensor_copy(acc[st2][:], pp[j][:])
                        else:
                            nc.vector.tensor_add(acc[st2][:], acc[st2][:],
                                                 pp[j][:])
                        if hc == HC - 1:
                            nc.sync.dma_start(
                                P_out[st2 * 128:(st2 + 1) * 128, :],
                                acc[st2][:])

    nc.compile()
    return nc


def _get_nc():
    if "nc" not in _cached:
        _cached["nc"] = _build()
    return _cached["nc"]


def _prep_inputs(inputs):
    x = np.asarray(inputs["x"], np.float32)
    fc = np.asarray(inputs["freqs_cos"], np.float32)   # [S, 32]
    fs = np.asarray(inputs["freqs_sin"], np.float32)
    w_cq = np.asarray(inputs["w_cq"], np.float32)
    w_dq_nope = np.asarray(inputs["w_dq_nope"], np.float32)
    w_dq_rope = np.asarray(inputs["w_dq_rope"], np.float32)
    w_ckv = np.asarray(inputs["w_ckv"], np.float32)
    w_dk_nope = np.asarray(inputs["w_dk_nope"], np.float32)
    w_dv = np.asarray(inputs["w_dv"], np.float32)
    w_krope = np.asarray(inputs["w_krope"], np.float32)
    w_proj = np.asarray(inputs["w_proj"], np.float32)
    qw = np.asarray(inputs["q_norm_w"], np.float32)
    kvw = np.asarray(inputs["kv_norm_w"], np.float32)

    perm = np.concatenate([np.arange(0, RD, 2), np.arange(1, RD, 2)])

    wlat = np.concatenate(
        [w_cq.T, w_ckv.T, w_krope[perm, :].T], axis=1).astype(BF)  # [D, LATW]
    wdqn = (w_dq_nope * qw[None, :]).reshape(H, ND, QR)
    wdqr = (w_dq_rope * qw[None, :]).reshape(H, RD, QR)[:, perm, :]
    wdkn = (w_dk_nope * kvw[None, :])
    wdv = (w_dv * kvw[None, :])
    wprojT = np.ascontiguousarray(w_proj.T).astype(BF)

    # all-heads decompress weights, column blocks ordered (pair, dest):
    # block (p*G+g2) covers heads (4*g2+2p, 4*g2+2p+1)
    head_order = []
    for p in range(2):
        for g2 in range(G):
            head_order += [4 * g2 + 2 * p, 4 * g2 + 2 * p + 1]
    wdqnA = np.ascontiguousarray(
        wdqn[head_order].reshape(H * ND, QR).T).astype(BF)
    wdqrA = np.ascontiguousarray(
        wdqr[head_order].reshape(H * RD, QR).T).astype(BF)

    # banded transposed rope tables [128, S]
    tabA = np.empty((128, S), np.float32)
    tabB = np.empty((128, S), np.float32)
    for band in range(4):
        rows = slice(band * 32, (band + 1) * 32)
        if band % 2 == 0:
            tabA[rows] = fc.T
            tabB[rows] = fs.T
        else:
            tabA[rows] = fs.T
            tabB[rows] = fc.T
    tabA = tabA.astype(BF)
    tabB = tabB.astype(BF)

    # band-combine selectors: rot = rotM1.T @ (x*tabA) + rotM2.T @ (x*tabB)
    # output 64-blocks are SWAPPED vs input blocks so that PSUM rows land on
    # the same base partitions as their kT destinations (even-head rope at
    # partitions 64:128, odd-head rope at 0:64).
    rotM1 = np.zeros((128, 128), np.float32)
    rotM2 = np.zeros((128, 128), np.float32)
    for b in range(2):
        for i in range(32):
            ei, oi = b * 64 + i, b * 64 + 32 + i            # input rows
            eo, oo = (1 - b) * 64 + i, (1 - b) * 64 + 32 + i  # output rows
            rotM1[ei, eo] = 1.0
            rotM1[oi, eo] = -1.0
            rotM2[ei, oo] = 1.0
            rotM2[oi, oo] = 1.0
    rotM1 = rotM1.astype(BF)
    rotM2 = rotM2.astype(BF)

    maskT = np.zeros((128, 128), np.float32)
    il, jl = np.tril_indices(128, -1)   # sq < sk  -> masked
    maskT[il, jl] = NEG

    in_maps = []
    for c in range(NC):
        b, g = divmod(c, G)
        hsl = slice(g * HC, (g + 1) * HC)
        ssl = slice(g * SC, (g + 1) * SC)
        xT_c = np.ascontiguousarray(x[b].T[:, ssl]).astype(BF)
        wdkn_c = np.ascontiguousarray(
            wdkn.reshape(H, ND, KVR)[hsl].reshape(HC * ND, KVR).T).astype(BF)
        wdv_c = np.ascontiguousarray(
            wdv.reshape(H, HD, KVR)[hsl].reshape(HC * HD, KVR).T).astype(BF)
        wproj_c = np.ascontiguousarray(wprojT[:, ssl])
        in_maps.append({
            "xT": xT_c,
            "wlat": wlat,
            "wdqn": wdqnA,
            "wdqr": wdqrA,
            "wdkn": wdkn_c,
            "wdv": wdv_c,
            "wproj": wproj_c,
            "tabA": np.ascontiguousarray(tabA[:, ssl]),
            "tabB": np.ascontiguousarray(tabB[:, ssl]),
            "tabKA": np.ascontiguousarray(
                np.concatenate([fc[ssl].T, fs[ssl].T], axis=0)).astype(BF),
            "tabKB": np.ascontiguousarray(
                np.concatenate([fs[ssl].T, fc[ssl].T], axis=0)).astype(BF),
            "rotM1": rotM1,
            "rotM2": rotM2,
            "mA": np.full((128, 1), 1.0 - b, np.float32),
            "mB": np.full((128, 1), float(b), np.float32),
            "maskT": maskT,
        })
    return in_maps


def _assemble(results):
    out = np.zeros((B, S, H * HD), np.float32)
    for c in range(NC):
        b, g = divmod(c, G)
        out[b, :, g * SC:(g + 1) * SC] = results[c]["out"]
    return out


def kernel(**inputs) -> np.ndarray:
    from concourse.bass_utils import run_bass_kernel_spmd
    nc = _get_nc()
    in_maps = _prep_inputs(inputs)
    res = run_bass_kernel_spmd(nc, in_maps, core_ids=list(range(NC)))
    return _assemble(res.results)
